# revision 38
# baseline (speedup 1.0000x reference)
"""Trainium2 Bass kernel for a 2-layer GAT (heads=1) + linear head + softmax.

Strategy (8 NeuronCores, graph/data parallel):
  - Nodes sharded across cores (12500 dst nodes each); edges partitioned by
    destination node so segment softmax / scatter stay local to a core.
  - Per layer, each core computes projected features for its node shard:
    table row = [h' (HID, bf16) | 1.0 | s_hi | s_lo | d_hi | d_lo]
    (s = h' @ a_src and d = h' @ a_dst, each split into two bf16 halves for
    ~f32 precision), then an AllGather replicates the full node table to
    every core's DRAM (halo exchange).
  - Edges are laid out in "slots": 8 slots per group, 16 groups per
    128-slot chunk, 24 chunks per 128-node destination block (384 groups =
    3 "gsum tiles" per block, padded -> identical program on every core).
    Slot 0 of every group holds the destination node's own table row: it is
    both the self-loop edge (valid=1 in the first group) and the carrier of
    d_dst for the group (duplicate carriers are zero-weighted via a host
    "valid" mask folded into the selector).
  - Main loop per layer: one big indirect-DMA gather of table rows per
    slot-chunk; d_dst per slot via one tiny matmul (mask16T x slot0-rows)
    broadcasting d across the 8 slot positions; per-edge attention logits
    e = leaky_relu(s_src + d_dst) and ex = exp(e) (no max-shift needed;
    logits are bounded), then a two-level matmul segment-reduction:
      level 1: ex*valid-carrying selector (static 16-label mask) x gathered
               rows -> per-group partial [sum(ex*h) | sum(ex)]
      level 2: is_equal(group-label, node-iota) selector x group partials
               -> per-node [numerator | denominator] accumulated in PSUM.
    Epilogue divides by the denominator (softmax normalization), adds bias,
    applies relu; layer 2 additionally applies the output head + softmax.

Performance notes (measured on HW):
  - The kernel is bound by the per-instruction cost of INDIRECT1D on
    GpSimd: ~1.1us Q7 SWDGE descriptor-gen (994ns fixed + 0.34ns/desc)
    plus ~310ns dispatch, i.e. ~1.43us per 128-row chunk regardless of
    row bytes or attached waits (verified with a dependency-free
    microbenchmark: 1426ns/gather). 2x2151 used chunks -> ~6.1ms floor.
  - Batching more rows per instruction is impossible in this environment:
    multi-column offset APs are mis-lowered by walrus (wrong descriptor
    count/elem size; [1,K] and [128,J>1] layouts crash or corrupt), and
    the batched InstDMAGatherAnt/extended-Q7 instructions are excluded
    from the bedrock image (running_on_bedrock()==True).
  - DMA queues run at ~20% occupancy; Tensor/Vector/Scalar fit entirely
    inside the gather-issue shadow. Head (prep+allgather) ~0.4ms, tail
    ~26us after the last gather.
"""

import math
import sys

import numpy as np

if "/opt/trn_rl_repo" not in sys.path:
    sys.path.insert(0, "/opt/trn_rl_repo")

import ml_dtypes

BF16 = ml_dtypes.bfloat16


# ---------------------------------------------------------------- config ---
class Cfg:
    def __init__(self, N, E, n_in=256, hid=128, ncls=3, ncores=8,
                 piece_blocks=5, gb=384):
        self.N, self.E = N, E
        self.N_IN, self.HID, self.NCLS = n_in, hid, ncls
        self.NCORES = ncores
        assert N % ncores == 0
        self.V = N // ncores                      # real dst nodes per core
        self.NB = math.ceil(self.V / 128)         # node blocks per core
        self.VPAD = self.NB * 128
        self.GB = gb                              # groups per block (padded)
        assert gb % 16 == 0
        self.CPB = gb // 16                       # chunks per block
        self.G = self.NB * self.GB                # groups per core
        assert self.G % 128 == 0
        self.NT = self.G // 128                   # gsum tiles per core (NB*3)
        self.TPB = self.GB // 128                 # gsum tiles per block
        assert self.GB % 128 == 0
        self.NCHUNK = self.NB * self.CPB
        self.SLOTS = self.NCHUNK * 128
        self.SENT = N                             # sentinel table row index
        self.ROW = hid + 5            # h | one | s_hi | s_lo | d_hi | d_lo
        self.RHS_W = hid + 1                      # matmul rhs width (h | one)
        # pieces: (block_start, nblocks); keep the final piece at 1 block so
        # the serial tail (last gather -> compute -> allgather/output) is
        # as short as possible
        self.pieces = []
        b = 0
        while b < self.NB:
            nb = min(piece_blocks, self.NB - b)
            self.pieces.append((b, nb))
            b += nb
        if self.pieces and self.pieces[-1][1] > 1:
            b0, nb = self.pieces[-1]
            self.pieces[-1] = (b0, nb - 1)
            self.pieces.append((b0 + nb - 1, 1))
        self.PIECE_BLOCKS = piece_blocks


DEFAULT_CFG = Cfg(N=100000, E=1600000)

# indirect-gather batching granularity: "piece" | "block" | "chunk"
GATHER_MODE = "chunk"
# split each table AllGather into two half-shard collectives (hidden behind
# prep/compute). CoreSim's Shared-DRAM model insists on a single writer per
# tensor, so set False when running under CoreSim.
SPLIT_AG = False
# HW INDIRECT1D consumes the index buffer channel-wrapped (flat index k read
# from partition k%128, column k//128) while the destination AP iterates
# partition-slowest; CoreSim pairs both in AP order. Host-permute indices for
# HW; set False when running under CoreSim.
HW_IDX_ORDER = True


def _wrap_idx_for_hw(src_slot, J):
    """Permute each J-column section so the HW channel-wrapped index read
    matches the destination's AP iteration order (partition-major)."""
    out = np.empty_like(src_slot)
    P, NCH = src_slot.shape
    k = np.arange(P * J)
    for c0 in range(0, NCH, J):
        out[k % P, c0 + k // P] = src_slot[k // J, c0 + (k % J)]
    return out


# ---------------------------------------------------- host preprocessing ---
def preprocess(cfg, edge_index):
    """Partition edges by destination core and build per-core slot layout.

    Slot 0 (partition = group label) of every group carries the destination
    node's own row: the self-loop edge for the node's first group, a
    zero-weighted d-carrier duplicate for subsequent groups. The remaining
    7 slots per group hold the node's other incoming edges.

    Returns per-core dict of index tensors (identical shapes on every core
    so one NEFF serves all 8).
    """
    import heapq

    src = np.concatenate([edge_index[0], np.arange(cfg.N, dtype=np.int32)])
    dst = np.concatenate([edge_index[1], np.arange(cfg.N, dtype=np.int32)])
    order = np.argsort(dst, kind="stable")
    src, dst = src[order].astype(np.int64), dst[order].astype(np.int64)
    core_of = dst // cfg.V
    bounds = np.searchsorted(core_of, np.arange(cfg.NCORES + 1))

    # ---- pass A: per-core group counts + block-balancing permutation ----
    # used_chunks is ceil(max-over-cores blk_tot / 16), so balancing group
    # counts across blocks (via a node permutation) trims gather chunks.
    perms, invs, ngrps, eds, ess = [], [], [], [], []
    caps = [128] * (cfg.NB - 1) + [cfg.V - 128 * (cfg.NB - 1)]
    for k in range(cfg.NCORES):
        lo, hi = bounds[k], bounds[k + 1]
        es = src[lo:hi]
        ed = dst[lo:hi] - k * cfg.V               # local dst, sorted
        deg = np.bincount(ed, minlength=cfg.V).astype(np.int64)
        # every node has >=1 incoming (the appended self loop)
        assert deg.min() >= 1
        degr = deg - 1
        ngrp = np.maximum(1, (degr + 6) // 7)
        # greedy LPT: heaviest nodes first into the lightest open block
        order_n = np.argsort(-ngrp, kind="stable")
        counts = [0] * cfg.NB
        members = [[] for _ in range(cfg.NB)]
        heap = [(0, b) for b in range(cfg.NB)]
        heapq.heapify(heap)
        for n in order_n:
            while True:
                s, b = heapq.heappop(heap)
                if counts[b] < caps[b]:
                    break
            members[b].append(n)
            counts[b] += 1
            if counts[b] < caps[b]:
                heapq.heappush(heap, (s + int(ngrp[n]), b))
        perm = np.concatenate([np.array(m, dtype=np.int64)
                               for m in members])      # new -> old
        inv = np.empty(cfg.V, np.int64)
        inv[perm] = np.arange(cfg.V)                   # old -> new
        perms.append(perm)
        invs.append(inv)
        ngrps.append(ngrp)
        eds.append(ed)
        ess.append(es)
    # old global id -> permuted table row id. Table layout is
    # [half0 of every core | half1 of every core] so the table AllGather can
    # be split into two half-shard collectives (see allgather_half).
    Vh = cfg.V // 2

    def _t_row(k, i):
        if not SPLIT_AG:
            return k * cfg.V + i
        return (i >= Vh) * (cfg.NCORES * Vh) + k * Vh + (i % Vh)

    g2t = np.concatenate([_t_row(k, invs[k]) for k in range(cfg.NCORES)])

    out = []
    for k in range(cfg.NCORES):
        perm, inv = perms[k], invs[k]
        # re-sort edges by permuted dst (stable keeps the appended self
        # edge last within each run)
        ed_new = inv[eds[k]]
        order2 = np.argsort(ed_new, kind="stable")
        es = g2t[ess[k][order2]]                  # src as table row ids
        ed = ed_new[order2]
        deg = np.bincount(ed, minlength=cfg.V).astype(np.int64)
        estart = np.zeros(cfg.V + 1, np.int64)
        np.cumsum(deg, out=estart[1:])
        degr = deg - 1
        ngrp = np.maximum(1, (degr + 6) // 7)

        # empty slots gather row 0 (harmless) and carry valid=0, so no
        # sentinel table row is needed (keeps the table single-writer =
        # AllGather only, required for Shared DRAM)
        src_slot = np.zeros((128, cfg.NCHUNK), np.int32)
        valid = np.zeros((128, cfg.NCHUNK), np.float32)
        glab = np.full(cfg.G, 1e9, np.float32)         # in-block node label

        nodes = np.arange(cfg.V, dtype=np.int64)
        blk = nodes // 128
        cumg = np.cumsum(ngrp)
        blk_start_node = blk * 128
        cumg_before_block = np.where(blk_start_node > 0,
                                     cumg[blk_start_node - 1], 0)
        gbase_n = (cumg - ngrp) - cumg_before_block
        blk_tot = np.zeros(cfg.NB, np.int64)
        np.add.at(blk_tot, blk, ngrp)
        assert blk_tot.max() <= cfg.GB, (
            f"core {k}: max groups/block {blk_tot.max()} > {cfg.GB}")
        # groups: labels + slot-0 self rows
        grp_node = np.repeat(nodes, ngrp)               # local node per group
        within = np.arange(len(grp_node), dtype=np.int64) - \
            np.repeat(cumg - ngrp, ngrp)                # 0..ngrp-1
        grel = gbase_n[grp_node] + within               # in-block group idx
        g_global = blk[grp_node] * cfg.GB + grel
        glab[g_global] = (grp_node % 128).astype(np.float32)
        lab_g = grel % 16
        chunk_g = blk[grp_node] * cfg.CPB + grel // 16
        src_slot[lab_g, chunk_g] = _t_row(k, grp_node).astype(np.int32)
        valid[lab_g[within == 0], chunk_g[within == 0]] = 1.0
        # non-self edges -> slots 1..7
        n_e = ed
        j_in = np.arange(len(ed), dtype=np.int64) - estart[n_e]
        keep = j_in < degr[n_e]     # drops the appended self edge (last)
        n_k = n_e[keep]
        j_k = j_in[keep]
        grel_e = gbase_n[n_k] + j_k // 7
        lab = grel_e % 16
        c = grel_e // 16
        p = lab + 16 * (1 + j_k % 7)
        chunk = blk[n_k] * cfg.CPB + c
        src_slot[p, chunk] = es[keep].astype(np.int32)
        valid[p, chunk] = 1.0
        # [p, T] layouts for the device
        glab_pt = glab.reshape(cfg.NT, 128).T.astype(np.float32).copy()
        out.append({
            "src_slot": src_slot,
            "glabel": glab_pt,
            "valid": valid.astype(BF16),
            "blk_tot": blk_tot.copy(),
            "perm": perm,
        })
    return out


def _used_chunks(cfg, pre):
    """Per-block chunk count actually carrying edges, maxed over cores (the
    NEFF is shared), rounded up to whole 16-group chunks."""
    mx = np.maximum.reduce([p["blk_tot"] for p in pre])
    return tuple(int(x) for x in np.minimum((mx + 15) // 16, cfg.CPB))


# ------------------------------------------------------------ bass build ---
def build_program(cfg, used_chunks=None):
    import concourse.bass as bass
    import concourse.bacc as bacc
    import concourse.mybir as mybir
    import concourse.tile as tile
    from concourse.bass import IndirectOffsetOnAxis, ds

    dt = mybir.dt
    F32, BF, I32 = dt.float32, dt.bfloat16, dt.int32
    AF = mybir.ActivationFunctionType
    OP = mybir.AluOpType
    HID, ROW, RHSW, NCLS = cfg.HID, cfg.ROW, cfg.RHS_W, cfg.NCLS

    if used_chunks is None:
        used_chunks = (cfg.CPB,) * cfg.NB
    nc = bacc.Bacc("TRN2", target_bir_lowering=False, debug=False,
                   enable_asserts=False, num_devices=cfg.NCORES)

    # ---- I/O ----
    xT = nc.dram_tensor("xT", [cfg.N_IN, cfg.VPAD], BF, kind="ExternalInput")
    W1 = nc.dram_tensor("W1", [cfg.N_IN, HID], F32, kind="ExternalInput")
    W2 = nc.dram_tensor("W2", [HID, HID], F32, kind="ExternalInput")
    Wo = nc.dram_tensor("Wo", [HID, NCLS], F32, kind="ExternalInput")
    a1s = nc.dram_tensor("a1s", [HID], F32, kind="ExternalInput")
    a1d = nc.dram_tensor("a1d", [HID], F32, kind="ExternalInput")
    a2s = nc.dram_tensor("a2s", [HID], F32, kind="ExternalInput")
    a2d = nc.dram_tensor("a2d", [HID], F32, kind="ExternalInput")
    b1 = nc.dram_tensor("b1", [HID], F32, kind="ExternalInput")
    b2 = nc.dram_tensor("b2", [HID], F32, kind="ExternalInput")
    bo = nc.dram_tensor("bo", [NCLS], F32, kind="ExternalInput")
    src_slot = nc.dram_tensor("src_slot", [128, cfg.NCHUNK], I32,
                              kind="ExternalInput")
    glab_in = nc.dram_tensor("glabel", [128, cfg.NT], F32,
                             kind="ExternalInput")
    valid_in = nc.dram_tensor("valid", [128, cfg.NCHUNK], BF,
                              kind="ExternalInput")
    out_t = nc.dram_tensor("out", [cfg.VPAD, NCLS], F32,
                           kind="ExternalOutput")

    # ---- inline constants ----
    ident_d = nc.inline_tensor(np.eye(128, dtype=np.float32), "ident")
    mask_np = (np.arange(128)[:, None] % 16 == np.arange(16)[None, :])
    mask_d = nc.inline_tensor(mask_np.astype(BF16), "mask16")
    # transposed 16-label mask: [16, 128], mask16T[l, p] = (p % 16 == l)
    mask_t_np = (np.arange(128)[None, :] % 16 == np.arange(16)[:, None])
    mask_t_d = nc.inline_tensor(mask_t_np.astype(BF16), "mask16T")
    iota_d = nc.inline_tensor(
        np.tile(np.arange(128, dtype=np.float32), (128, 1)), "iota2d")

    groups = [list(range(cfg.NCORES))]

    with tile.TileContext(nc, num_cores=cfg.NCORES) as tc:
        with (
            tc.tile_pool(name="const", bufs=1) as cp,
            tc.tile_pool(name="work", bufs=5) as wp,
            tc.tile_pool(name="gsum", bufs=8) as sp,
            tc.tile_pool(name="psum", bufs=2, space="PSUM") as pp,
            tc.tile_pool(name="dram", bufs=1, space="DRAM") as dp,
        ):
            # ======== constants to SBUF ========
            ident = cp.tile([128, 128], F32, tag="ident")
            nc.sync.dma_start(ident[:], ident_d[:, :])
            mask16 = cp.tile([128, 16], BF, tag="mask16")
            nc.sync.dma_start(mask16[:], mask_d[:, :])
            mask16t = cp.tile([16, 128], BF, tag="mask16t")
            nc.sync.dma_start(mask16t[:], mask_t_d[:, :])
            iota2 = cp.tile([128, 128], F32, tag="iota2")
            nc.sync.dma_start(iota2[:], iota_d[:, :])
            srcsb = cp.tile([128, cfg.NCHUNK], I32, tag="srcsb")
            nc.sync.dma_start(srcsb[:], src_slot[:, :])
            glabsb = cp.tile([128, cfg.NT], F32, tag="glabsb")
            nc.sync.dma_start(glabsb[:], glab_in[:, :])
            validsb = cp.tile([128, cfg.NCHUNK], BF, tag="validsb")
            nc.sync.dma_start(validsb[:], valid_in[:, :])
            wosb = cp.tile([128, NCLS], F32, tag="wosb")
            nc.sync.dma_start(wosb[:], Wo[:, :])
            b1r = cp.tile([128, HID], F32, tag="b1r")
            nc.sync.dma_start(b1r[:], b1[None, :].to_broadcast([128, HID]))
            b2r = cp.tile([128, HID], F32, tag="b2r")
            nc.sync.dma_start(b2r[:], b2[None, :].to_broadcast([128, HID]))
            bor = cp.tile([128, NCLS], F32, tag="bor")
            nc.sync.dma_start(bor[:], bo[None, :].to_broadcast([128, NCLS]))

            def make_rhs(Wd, asd_s, asd_d, nchunks, tagbase, out_bf=False):
                """rhs tiles [128, HID+2] = [W chunk | W@a_src | W@a_dst]."""
                asd = cp.tile([128, 2], F32, tag=tagbase + "_asd")
                nc.sync.dma_start(asd[:, 0:1], asd_s[:, None])
                nc.sync.dma_start(asd[:, 1:2], asd_d[:, None])
                tiles = []
                for c in range(nchunks):
                    rt = cp.tile([128, HID + 2], F32, tag=f"{tagbase}_{c}")
                    nc.sync.dma_start(rt[:, 0:HID],
                                      Wd[c * 128:(c + 1) * 128, :])
                    tp = pp.tile([128, 128], F32, tag="mm")
                    nc.tensor.transpose(tp[:], rt[:, 0:HID], ident[:])
                    wt = wp.tile([128, 128], F32, tag="wt")
                    nc.scalar.activation(wt[:], tp[:], AF.Copy)
                    sp2 = pp.tile([128, 2], F32, tag="mm")
                    nc.tensor.matmul(sp2[:], lhsT=wt[:], rhs=asd[:],
                                     start=True, stop=True)
                    nc.vector.tensor_copy(rt[:, HID:HID + 2], sp2[:])
                    if out_bf:
                        rtb = cp.tile([128, HID + 2], BF, tag=f"{tagbase}b{c}")
                        nc.vector.tensor_copy(rtb[:], rt[:])
                        tiles.append(rtb)
                    else:
                        tiles.append(rt)
                return tiles

            # layer-1 projection in bf16 (x is ~N(0,1); f32 PSUM accumulate
            # keeps the error well inside budget) -> half the xT DMA and 2x
            # the prep matmul rate
            rhs1 = make_rhs(W1, a1s, a1d, cfg.N_IN // 128, "rhs1",
                            out_bf=True)
            rhs2 = make_rhs(W2, a2s, a2d, 1, "rhs2")

            # persistent selector tiles: per chunk-pair layout is
            # [realA(16) | zeros(16) | realB(16)] so that the [*,32] lhsT
            # slice of either chunk has true zeros in its other half.
            # memset once; per-piece builds only touch the real columns.
            SELW = 48 * (cfg.PIECE_BLOCKS * cfg.CPB // 2)
            GTW = cfg.ROW * cfg.PIECE_BLOCKS * cfg.CPB
            # zero-fill via broadcast DMA (keeps GpSimd free for the
            # serialized indirect gathers, its real bottleneck)
            zrow_d = nc.inline_tensor(
                np.zeros((1, max(SELW, GTW)), dtype=BF16), "zrow")
            selA = cp.tile([128, SELW], BF, tag="selA")
            selB = cp.tile([128, SELW], BF, tag="selB")
            nc.sync.dma_start(selA[:],
                              zrow_d[0:1, 0:SELW].to_broadcast([128, SELW]))
            nc.sync.dma_start(selB[:],
                              zrow_d[0:1, 0:SELW].to_broadcast([128, SELW]))
            gtA = cp.tile([128, GTW], BF, tag="gtA")
            gtB = cp.tile([128, GTW], BF, tag="gtB")
            nc.sync.dma_start(gtA[:],
                              zrow_d[0:1, 0:GTW].to_broadcast([128, GTW]))
            nc.sync.dma_start(gtB[:],
                              zrow_d[0:1, 0:GTW].to_broadcast([128, GTW]))
            ones1 = cp.tile([128, 1], F32, tag="ones1")
            nc.gpsimd.memset(ones1[:], 1.0)

            # DRAM scratch (tables are AllGather outputs -> Shared address
            # space, else the collective bounces through an extra copy)
            tables = [dp.tile([cfg.N, ROW], BF, name=f"table{i}",
                              tag=f"table{i}", addr_space="Shared")
                      for i in range(2)]
            shards = [dp.tile([cfg.V, ROW], BF, name=f"shard{i}",
                              tag=f"shard{i}") for i in range(2)]

            # ======== per-layer table prep ========
            def build_rows(ppre, b, shard):
                """ppre: psum [128, HID+2] = [h' | s | d] for block b.

                Row layout: [h | one | s_hi d_hi | s_lo d_lo] so both hi
                halves are one scalar copy and both lo halves one vector
                subtract (halves the per-block dependency chain)."""
                rows = wp.tile([128, ROW], BF, tag="rows")
                nc.vector.tensor_copy(rows[:, 0:HID], ppre[:, 0:HID])
                nc.scalar.activation(rows[:, HID:HID + 1], ones1[:], AF.Copy)
                nc.scalar.activation(rows[:, HID + 1:HID + 3],
                                     ppre[:, HID:HID + 2], AF.Copy)
                nc.vector.tensor_tensor(rows[:, HID + 3:HID + 5],
                                        ppre[:, HID:HID + 2],
                                        rows[:, HID + 1:HID + 3],
                                        op=OP.subtract)
                nrow = min(128, cfg.V - b * 128)
                nc.sync.dma_start(shard[b * 128:b * 128 + nrow, :],
                                  rows[0:nrow, :])

            # split allgathers: table layout is [half0 of every core |
            # half1 of every core] so each half-shard allgather can fire as
            # soon as its rows are written, hiding the collective behind
            # prep (layer 1) / main-loop compute (layer 2).
            Vh = cfg.V // 2
            AG_SPLIT_BLOCK = Vh // 128          # shard block covering row Vh

            def allgather_half(li, h):
                if not SPLIT_AG:
                    if h == 1:
                        nc.gpsimd.collective_compute(
                            "AllGather", mybir.AluOpType.bypass,
                            replica_groups=groups,
                            ins=[shards[li][:, :].opt()],
                            outs=[tables[li][0:cfg.N, :].opt()],
                        )
                    return
                lo = h * Vh
                out_lo = h * cfg.NCORES * Vh
                nc.gpsimd.collective_compute(
                    "AllGather", mybir.AluOpType.bypass,
                    replica_groups=groups,
                    ins=[shards[li][lo:lo + Vh, :].opt()],
                    outs=[tables[li][out_lo:out_lo + cfg.NCORES * Vh,
                                     :].opt()],
                )

            def prep_layer1():
                for b in range(cfg.NB):
                    ppre = pp.tile([128, HID + 2], F32, tag="mm")
                    for c in range(cfg.N_IN // 128):
                        xt = wp.tile([128, 128], BF, tag="xt")
                        nc.sync.dma_start(
                            xt[:],
                            xT[c * 128:(c + 1) * 128,
                               b * 128:(b + 1) * 128])
                        nc.tensor.matmul(ppre[:], lhsT=xt[:], rhs=rhs1[c][:],
                                         start=(c == 0),
                                         stop=(c == cfg.N_IN // 128 - 1))
                    build_rows(ppre, b, shards[0])
                    if b == AG_SPLIT_BLOCK:
                        allgather_half(0, 0)

            # ======== main per-layer loop ========
            def main_layer(li, epilogue, mid_cb=None):
                table = tables[li]
                PB = cfg.PIECE_BLOCKS
                for pi, (b0, nb) in enumerate(cfg.pieces):
                    NCh = nb * cfg.CPB
                    # feature gather. SWDGE cost is ~994ns fixed per
                    # instruction + 0.34ns/descriptor, so batching chunks
                    # into one indirect DMA cuts GpSimd issue time. Unused
                    # tail chunks gather row 0: zero-weighted junk.
                    gt = gtA if pi % 2 == 0 else gtB
                    if GATHER_MODE == "hwloop":
                        # hardware loop: one decoded indirect-DMA replayed
                        # NCh times with register-stepped offsets (cuts the
                        # per-instruction sequencer fetch/dispatch cost)
                        with tc.For_i(0, NCh) as j:
                            nc.gpsimd.indirect_dma_start(
                                out=gt[:, ds(j * ROW, ROW)], out_offset=None,
                                in_=table[:, :],
                                in_offset=IndirectOffsetOnAxis(
                                    ap=srcsb[:, ds(b0 * cfg.CPB + j, 1)],
                                    axis=0))
                    elif GATHER_MODE == "piece":
                        off = srcsb[:, b0 * cfg.CPB:(b0 + nb) * cfg.CPB]
                        dstv = gt[:, 0:ROW * NCh].rearrange(
                            "p (j r) -> p j r", r=ROW)
                        nc.gpsimd.indirect_dma_start(
                            out=dstv, out_offset=None,
                            in_=table[:, :],
                            in_offset=IndirectOffsetOnAxis(ap=off, axis=0))
                    elif GATHER_MODE == "block":
                        for bb in range(nb):
                            j0 = bb * cfg.CPB
                            off = srcsb[:, (b0 + bb) * cfg.CPB:
                                        (b0 + bb + 1) * cfg.CPB]
                            dstv = gt[:, ROW * j0:ROW * (j0 + cfg.CPB)] \
                                .rearrange("p (j r) -> p j r", r=ROW)
                            nc.gpsimd.indirect_dma_start(
                                out=dstv, out_offset=None,
                                in_=table[:, :],
                                in_offset=IndirectOffsetOnAxis(ap=off, axis=0))
                    else:  # per-chunk (original)
                        for j in range(NCh):
                            if (j % cfg.CPB) >= used_chunks[b0 + j // cfg.CPB]:
                                continue
                            nc.gpsimd.indirect_dma_start(
                                out=gt[:, ROW * j:ROW * (j + 1)],
                                out_offset=None,
                                in_=table[:, :],
                                in_offset=IndirectOffsetOnAxis(
                                    ap=srcsb[:, b0 * cfg.CPB + j:
                                             b0 * cfg.CPB + j + 1],
                                    axis=0))
                    gv = gt[:, 0:ROW * NCh].rearrange(
                        "p (j c) -> p j c", c=ROW)
                    # d_dst per slot: broadcast slot-0 rows' d across the 8
                    # slot positions with one 16-contraction matmul pair
                    pd = pp.tile([128, PB * cfg.CPB], F32, tag="pd")
                    gvt = gt[0:16, 0:ROW * NCh].rearrange(
                        "p (j c) -> p j c", c=ROW)
                    nc.tensor.matmul(
                        pd[:, 0:NCh], lhsT=mask16t[:],
                        rhs=gvt[:, :, HID + 2:HID + 3].rearrange(
                            "p j c -> p (j c)"),
                        start=True, stop=False)
                    nc.tensor.matmul(
                        pd[:, 0:NCh], lhsT=mask16t[:],
                        rhs=gvt[:, :, HID + 4:HID + 5].rearrange(
                            "p j c -> p (j c)"),
                        start=False, stop=True)
                    # phase A: ex = exp(leaky_relu(s_src + d_dst)) * valid
                    sf = wp.tile([128, PB * cfg.CPB], F32, tag="sf")
                    sfv = sf[:, 0:NCh].rearrange("p j -> p j ()")
                    nc.vector.tensor_tensor(
                        sfv, gv[:, :, HID + 1:HID + 2],
                        gv[:, :, HID + 3:HID + 4], op=OP.add)
                    ep = wp.tile([128, PB * cfg.CPB], F32, tag="ep")
                    nc.vector.tensor_tensor(ep[:, 0:NCh], sf[:, 0:NCh],
                                            pd[:, 0:NCh], op=OP.add)
                    es = wp.tile([128, PB * cfg.CPB], F32, tag="es")
                    nc.vector.tensor_scalar_mul(es[:, 0:NCh], ep[:, 0:NCh],
                                                0.2)
                    el = wp.tile([128, PB * cfg.CPB], F32, tag="el")
                    nc.vector.tensor_tensor(el[:, 0:NCh], ep[:, 0:NCh],
                                            es[:, 0:NCh], op=OP.max)
                    exf = wp.tile([128, PB * cfg.CPB], F32, tag="exf")
                    nc.scalar.activation(exf[:, 0:NCh], el[:, 0:NCh], AF.Exp)
                    exb = wp.tile([128, PB * cfg.CPB], BF, tag="exb")
                    nc.vector.tensor_tensor(
                        exb[:, 0:NCh], exf[:, 0:NCh],
                        validsb[:, b0 * cfg.CPB:b0 * cfg.CPB + NCh],
                        op=OP.mult)
                    # selector build: even chunks -> cols [48a, 48a+16),
                    # odd chunks -> cols [48a+32, 48a+48)
                    sel1 = selA if (b0 // cfg.PIECE_BLOCKS) % 2 == 0 else selB
                    npair = NCh // 2
                    exv = exb[:, 0:NCh].rearrange("p (a u) -> p a u", u=2)
                    maskv = mask16[:].rearrange("p l -> p () l") \
                        .to_broadcast([128, npair, 16])
                    selv = sel1[:, 0:48 * npair].rearrange(
                        "p (a w) -> p a w", w=48)
                    nc.vector.tensor_tensor(
                        selv[:, :, 0:16],
                        exv[:, :, 0:1].to_broadcast([128, npair, 16]),
                        maskv, op=OP.mult)
                    nc.vector.tensor_tensor(
                        selv[:, :, 32:48],
                        exv[:, :, 1:2].to_broadcast([128, npair, 16]),
                        maskv, op=OP.mult)
                    # level 1 + level 2
                    for bb in range(nb):
                        b = b0 + bb
                        pl2 = pp.tile([128, RHSW], F32, tag="l2")
                        for t in range(cfg.TPB):
                            pl1 = pp.tile([128, RHSW], F32, tag="l1")
                            for al in range(4):
                                for u in range(2):
                                    j = bb * cfg.CPB + t * 8 + 2 * al + u
                                    A = j // 2
                                    nc.tensor.matmul(
                                        pl1[32 * al:32 * al + 32, :],
                                        lhsT=sel1[:, 48 * A + 16 * u:
                                                  48 * A + 16 * u + 32],
                                        rhs=gt[:, ROW * j:ROW * j + RHSW],
                                        start=(u == 0), stop=(u == 1),
                                        tile_position=(0, 32 * al))
                            gs = sp.tile([128, RHSW], F32, tag="gsum")
                            nc.scalar.activation(gs[:], pl1[:], AF.Copy)
                            T = b * cfg.TPB + t
                            l2s = wp.tile([128, 128], F32, tag="l2s")
                            nc.vector.tensor_tensor(
                                l2s[:],
                                glabsb[:, T:T + 1].to_broadcast([128, 128]),
                                iota2[:], op=OP.is_equal)
                            nc.tensor.matmul(pl2[:], lhsT=l2s[:], rhs=gs[:],
                                             start=(t == 0),
                                             stop=(t == cfg.TPB - 1))
                        epilogue(b, pl2)
                    if mid_cb is not None:
                        mid_cb(pi)

            def epi_norm(pl2, brow):
                den = wp.tile([128, 1], F32, tag="den")
                nc.vector.tensor_scalar_max(den[:], pl2[:, HID:HID + 1],
                                            1e-30)
                rec = wp.tile([128, 1], F32, tag="rec")
                nc.vector.reciprocal(rec[:], den[:])
                hb = wp.tile([128, HID], F32, tag="hb")
                nc.vector.tensor_scalar_mul(hb[:], pl2[:, 0:HID],
                                            rec[:, 0:1])
                nc.vector.tensor_tensor(hb[:], hb[:], brow[:], op=OP.add)
                return hb

            def epilogue1(b, pl2):
                # fused layer-2 prep: h2 -> [h'|s|d] rows for block b, so
                # only the AllGather sits between the two main loops
                hb = epi_norm(pl2, b1r)
                h2 = wp.tile([128, HID], F32, tag="h2")
                nc.scalar.activation(h2[:], hb[:], AF.Relu)
                tp = pp.tile([128, 128], F32, tag="mm")
                nc.tensor.transpose(tp[:], h2[:], ident[:])
                h2T = wp.tile([128, 128], F32, tag="h2T")
                nc.scalar.activation(h2T[:], tp[:], AF.Copy)
                ppre = pp.tile([128, HID + 2], F32, tag="mm")
                nc.tensor.matmul(ppre[:], lhsT=h2T[:], rhs=rhs2[0][:],
                                 start=True, stop=True)
                build_rows(ppre, b, shards[1])

            def epilogue2(b, pl2):
                hb = epi_norm(pl2, b2r)
                h3 = wp.tile([128, HID], F32, tag="h3")
                nc.scalar.activation(h3[:], hb[:], AF.Relu)
                tp = pp.tile([128, 128], F32, tag="mm")
                nc.tensor.transpose(tp[:], h3[:], ident[:])
                h3T = wp.tile([128, 128], F32, tag="h3T")
                nc.scalar.activation(h3T[:], tp[:], AF.Copy)
                po = pp.tile([128, NCLS], F32, tag="mm")
                nc.tensor.matmul(po[:], lhsT=h3T[:], rhs=wosb[:],
                                 start=True, stop=True)
                lg = wp.tile([128, NCLS], F32, tag="lg")
                nc.vector.tensor_tensor(lg[:], po[:], bor[:], op=OP.add)
                mx = wp.tile([128, 1], F32, tag="mx")
                nc.vector.tensor_reduce(mx[:], lg[:],
                                        axis=mybir.AxisListType.X, op=OP.max)
                lgs = wp.tile([128, NCLS], F32, tag="lgs")
                nc.vector.tensor_scalar(lgs[:], lg[:], mx[:, 0:1], None,
                                        op0=OP.subtract)
                pe = wp.tile([128, NCLS], F32, tag="pe")
                rs = wp.tile([128, 1], F32, tag="rs")
                nc.scalar.activation(pe[:], lgs[:], AF.Exp,
                                     accum_out=rs[:, 0:1])
                rr = wp.tile([128, 1], F32, tag="rr")
                nc.vector.reciprocal(rr[:], rs[:])
                ot = wp.tile([128, NCLS], F32, tag="ot")
                nc.vector.tensor_scalar_mul(ot[:], pe[:], rr[:, 0:1])
                nc.sync.dma_start(out_t[b * 128:(b + 1) * 128, :], ot[:])

            # ======== schedule ========
            # ag half-0s fire early (inside prep_layer1 / mid main-loop) so
            # only the half-1 collectives sit on the critical path.
            AG1_PIECE = max(0, min(len(cfg.pieces) - 2,
                                   (AG_SPLIT_BLOCK // cfg.PIECE_BLOCKS) + 3))

            def mid1(pi):
                if pi == AG1_PIECE:
                    allgather_half(1, 0)

            prep_layer1()
            allgather_half(0, 1)
            main_layer(0, epilogue1, mid_cb=mid1)
            allgather_half(1, 1)
            main_layer(1, epilogue2)

    nc.compile()
    return nc


# ------------------------------------------------------------------ run ---
_PROG_CACHE = {}


def _get_program(cfg, used_chunks=None):
    key = (cfg.N, cfg.E, cfg.NCORES, used_chunks)
    if key not in _PROG_CACHE:
        _PROG_CACHE[key] = build_program(cfg, used_chunks)
    return _PROG_CACHE[key]


_PRE_CACHE = {}


def run(cfg, inputs, trace=False, tmpdir=None):
    from concourse.bass_utils import run_bass_kernel_spmd

    ei = np.asarray(inputs["edge_index"])
    pkey = (cfg.N, cfg.E, int(ei[0, :16].sum()), int(ei[1, -16:].sum()))
    if pkey not in _PRE_CACHE:
        _PRE_CACHE[pkey] = preprocess(cfg, ei)
    pre = _PRE_CACHE[pkey]
    nc = _get_program(cfg, _used_chunks(cfg, pre))
    in_maps = _make_in_maps(cfg, inputs, pre)
    kw = {}
    if trace:
        kw = dict(trace=True, tmpdir=tmpdir)
    res = run_bass_kernel_spmd(nc, in_maps, core_ids=list(range(cfg.NCORES)),
                               **kw)
    outs = []
    for k in range(cfg.NCORES):
        o = res.results[k]["out"][:cfg.V]
        oo = np.empty_like(o)
        oo[pre[k]["perm"]] = o                   # undo block-balancing perm
        outs.append(oo)
    full = np.concatenate(outs, axis=0).astype(np.float32)
    return full, res


def timed_run(cfg, inputs, iters=3):
    """Time device execution with device-resident inputs (axon transfer
    excluded). Returns (best_seconds, outputs_of_last_iter)."""
    import time

    import jax
    from jax.sharding import Mesh, PartitionSpec
    from jax.experimental.shard_map import shard_map
    import concourse.mybir as mybir
    from concourse import bass2jax
    from concourse.bass2jax import _bass_exec_p, partition_id_tensor

    bass2jax.install_neuronx_cc_hook()
    ei = np.asarray(inputs["edge_index"])
    pkey = (cfg.N, cfg.E, int(ei[0, :16].sum()), int(ei[1, -16:].sum()))
    if pkey not in _PRE_CACHE:
        _PRE_CACHE[pkey] = preprocess(cfg, ei)
    pre = _PRE_CACHE[pkey]
    nc = _get_program(cfg, _used_chunks(cfg, pre))
    in_maps = _make_in_maps(cfg, inputs, pre)

    partition_name = (nc.partition_id_tensor.name
                      if nc.partition_id_tensor else None)
    in_names, out_names, out_avals, zero_outs = [], [], [], []
    for alloc in nc.m.functions[0].allocations:
        if not isinstance(alloc, mybir.MemoryLocationSet):
            continue
        name = alloc.memorylocations[0].name
        if alloc.kind == "ExternalInput":
            if name != partition_name:
                in_names.append(name)
        elif alloc.kind == "ExternalOutput":
            out_names.append(name)
            shape = tuple(alloc.tensor_shape)
            dtype = mybir.dt.np(alloc.dtype)
            out_avals.append(jax.core.ShapedArray(shape, dtype))
            zero_outs.append(np.zeros(shape, dtype))
    n_params = len(in_names)
    n_outs = len(out_avals)
    all_in_names = list(in_names) + list(out_names)
    if partition_name is not None:
        all_in_names.append(partition_name)
    donate = tuple(range(n_params, n_params + n_outs))

    def _body(*args):
        operands = list(args)
        if partition_name is not None:
            operands.append(partition_id_tensor())
        outs = _bass_exec_p.bind(
            *operands, out_avals=tuple(out_avals),
            in_names=tuple(all_in_names), out_names=tuple(out_names),
            lowering_input_output_aliases=(),
            sim_require_finite=True, sim_require_nnan=True, nc=nc)
        return tuple(outs)

    devices = jax.devices()[:cfg.NCORES]
    mesh = Mesh(np.asarray(devices), ("core",))
    in_specs = (PartitionSpec("core"),) * (n_params + n_outs)
    out_specs = (PartitionSpec("core"),) * len(out_names)
    sharded = jax.jit(
        shard_map(_body, mesh=mesh, in_specs=in_specs, out_specs=out_specs,
                  check_rep=False),
        donate_argnums=donate, keep_unused=True)
    concat_in = [
        np.concatenate([np.asarray(in_maps[c][nm]) for c in range(cfg.NCORES)],
                       axis=0)
        for nm in in_names]
    sharding = jax.sharding.NamedSharding(mesh, PartitionSpec("core"))
    dev_in = [jax.device_put(a, sharding) for a in concat_in]
    times = []
    out_arrs = None
    for _ in range(iters):
        zo = [jax.device_put(
            np.zeros((cfg.NCORES * z.shape[0], *z.shape[1:]), z.dtype),
            sharding) for z in zero_outs]
        jax.block_until_ready(zo)
        t0 = time.time()
        out_arrs = sharded(*dev_in, *zo)
        jax.block_until_ready(out_arrs)
        times.append(time.time() - t0)
    oi = out_names.index("out")
    full = np.asarray(out_arrs[oi]).reshape(cfg.NCORES, cfg.VPAD, cfg.NCLS)
    outs = []
    for c in range(cfg.NCORES):
        o = full[c, :cfg.V]
        oo = np.empty_like(o)
        oo[pre[c]["perm"]] = o
        outs.append(oo)
    out = np.concatenate(outs, axis=0)
    return min(times), out.astype(np.float32)


def _make_in_maps(cfg, inputs, pre):
    x = np.asarray(inputs["x"], np.float32)
    common = {
        "W1": np.asarray(inputs["W1"], np.float32),
        "W2": np.asarray(inputs["W2"], np.float32),
        "Wo": np.asarray(inputs["Wo"], np.float32),
        "a1s": np.asarray(inputs["a1_src"], np.float32),
        "a1d": np.asarray(inputs["a1_dst"], np.float32),
        "a2s": np.asarray(inputs["a2_src"], np.float32),
        "a2d": np.asarray(inputs["a2_dst"], np.float32),
        "b1": np.asarray(inputs["b1"], np.float32),
        "b2": np.asarray(inputs["b2"], np.float32),
        "bo": np.asarray(inputs["bo"], np.float32),
    }
    in_maps = []
    for k in range(cfg.NCORES):
        xs = x[k * cfg.V:(k + 1) * cfg.V][pre[k]["perm"]]
        xT = np.zeros((cfg.N_IN, cfg.VPAD), BF16)
        xT[:, :cfg.V] = xs.T.astype(BF16)
        m = dict(common)
        m["xT"] = np.ascontiguousarray(xT)
        if GATHER_MODE == "block" and HW_IDX_ORDER:
            m["src_slot"] = _wrap_idx_for_hw(pre[k]["src_slot"], cfg.CPB)
        else:
            m["src_slot"] = pre[k]["src_slot"]
        m["glabel"] = pre[k]["glabel"]
        m["valid"] = pre[k]["valid"]
        in_maps.append(m)
    return in_maps


def kernel(**inputs):
    cfg = DEFAULT_CFG
    full, _ = run(cfg, inputs, trace=False)
    return full



# revision 42
# speedup vs baseline: 1.0087x; 1.0087x over previous
"""Trainium2 Bass kernel for a 2-layer GAT (heads=1) + linear head + softmax.

Strategy (8 NeuronCores, graph/data parallel):
  - Nodes sharded across cores (12500 dst nodes each); edges partitioned by
    destination node so segment softmax / scatter stay local to a core.
  - Per layer, each core computes projected features for its node shard:
    table row = [h' (HID, bf16) | 1.0 | s_hi | s_lo | d_hi | d_lo]
    (s = h' @ a_src and d = h' @ a_dst, each split into two bf16 halves for
    ~f32 precision), then an AllGather replicates the full node table to
    every core's DRAM (halo exchange).
  - Edges are laid out in "slots": 8 slots per group, 16 groups per
    128-slot chunk, 24 chunks per 128-node destination block (384 groups =
    3 "gsum tiles" per block, padded -> identical program on every core).
    Slot 0 of every group holds the destination node's own table row: it is
    both the self-loop edge (valid=1 in the first group) and the carrier of
    d_dst for the group (duplicate carriers are zero-weighted via a host
    "valid" mask folded into the selector).
  - Main loop per layer: one big indirect-DMA gather of table rows per
    slot-chunk; d_dst per slot via one tiny matmul (mask16T x slot0-rows)
    broadcasting d across the 8 slot positions; per-edge attention logits
    e = leaky_relu(s_src + d_dst) and ex = exp(e) (no max-shift needed;
    logits are bounded), then a two-level matmul segment-reduction:
      level 1: ex*valid-carrying selector (static 16-label mask) x gathered
               rows -> per-group partial [sum(ex*h) | sum(ex)]
      level 2: is_equal(group-label, node-iota) selector x group partials
               -> per-node [numerator | denominator] accumulated in PSUM.
    Epilogue divides by the denominator (softmax normalization), adds bias,
    applies relu; layer 2 additionally applies the output head + softmax.

Performance notes (measured on HW):
  - The kernel is bound by the per-instruction cost of INDIRECT1D on
    GpSimd: ~1.1us Q7 SWDGE descriptor-gen (994ns fixed + 0.34ns/desc)
    plus ~310ns dispatch, i.e. ~1.43us per 128-row chunk regardless of
    row bytes or attached waits (verified with a dependency-free
    microbenchmark: 1426ns/gather). 2x2151 used chunks -> ~6.1ms floor.
  - Batching more rows per instruction is impossible in this environment:
    multi-column offset APs are mis-lowered by walrus (wrong descriptor
    count/elem size; [1,K] and [128,J>1] layouts crash or corrupt), and
    the batched InstDMAGatherAnt/extended-Q7 instructions are excluded
    from the bedrock image (running_on_bedrock()==True).
  - DMA queues run at ~20% occupancy; Tensor/Vector/Scalar fit entirely
    inside the gather-issue shadow. Head (prep+allgather) ~0.4ms, tail
    ~26us after the last gather.
"""

import math
import sys

import numpy as np

if "/opt/trn_rl_repo" not in sys.path:
    sys.path.insert(0, "/opt/trn_rl_repo")

import ml_dtypes

BF16 = ml_dtypes.bfloat16


# ---------------------------------------------------------------- config ---
class Cfg:
    def __init__(self, N, E, n_in=256, hid=128, ncls=3, ncores=8,
                 piece_blocks=5, gb=384):
        self.N, self.E = N, E
        self.N_IN, self.HID, self.NCLS = n_in, hid, ncls
        self.NCORES = ncores
        assert N % ncores == 0
        self.V = N // ncores                      # real dst nodes per core
        self.NB = math.ceil(self.V / 128)         # node blocks per core
        self.VPAD = self.NB * 128
        self.GB = gb                              # groups per block (padded)
        assert gb % 16 == 0
        self.CPB = gb // 16                       # chunks per block
        self.G = self.NB * self.GB                # groups per core
        assert self.G % 128 == 0
        self.NT = self.G // 128                   # gsum tiles per core (NB*3)
        self.TPB = self.GB // 128                 # gsum tiles per block
        assert self.GB % 128 == 0
        self.NCHUNK = self.NB * self.CPB
        self.SLOTS = self.NCHUNK * 128
        self.SENT = N                             # sentinel table row index
        self.ROW = hid + 5            # h | one | s_hi | s_lo | d_hi | d_lo
        self.RHS_W = hid + 1                      # matmul rhs width (h | one)
        # pieces: (block_start, nblocks); keep the final piece at 1 block so
        # the serial tail (last gather -> compute -> allgather/output) is
        # as short as possible
        self.pieces = []
        b = 0
        while b < self.NB:
            nb = min(piece_blocks, self.NB - b)
            self.pieces.append((b, nb))
            b += nb
        if self.pieces and self.pieces[-1][1] > 1:
            b0, nb = self.pieces[-1]
            self.pieces[-1] = (b0, nb - 1)
            self.pieces.append((b0 + nb - 1, 1))
        self.PIECE_BLOCKS = piece_blocks


DEFAULT_CFG = Cfg(N=100000, E=1600000)

# indirect-gather batching granularity: "piece" | "block" | "chunk"
GATHER_MODE = "chunk"
# split each table AllGather into two half-shard collectives (hidden behind
# prep/compute). CoreSim's Shared-DRAM model insists on a single writer per
# tensor, so set False when running under CoreSim.
SPLIT_AG = False
# HW INDIRECT1D consumes the index buffer channel-wrapped (flat index k read
# from partition k%128, column k//128) while the destination AP iterates
# partition-slowest; CoreSim pairs both in AP order. Host-permute indices for
# HW; set False when running under CoreSim.
HW_IDX_ORDER = True


def _wrap_idx_for_hw(src_slot, J):
    """Permute each J-column section so the HW channel-wrapped index read
    matches the destination's AP iteration order (partition-major)."""
    out = np.empty_like(src_slot)
    P, NCH = src_slot.shape
    k = np.arange(P * J)
    for c0 in range(0, NCH, J):
        out[k % P, c0 + k // P] = src_slot[k // J, c0 + (k % J)]
    return out


# ---------------------------------------------------- host preprocessing ---
def preprocess(cfg, edge_index):
    """Partition edges by destination core and build per-core slot layout.

    Slot 0 (partition = group label) of every group carries the destination
    node's own row: the self-loop edge for the node's first group, a
    zero-weighted d-carrier duplicate for subsequent groups. The remaining
    7 slots per group hold the node's other incoming edges.

    Returns per-core dict of index tensors (identical shapes on every core
    so one NEFF serves all 8).
    """
    import heapq

    src = np.concatenate([edge_index[0], np.arange(cfg.N, dtype=np.int32)])
    dst = np.concatenate([edge_index[1], np.arange(cfg.N, dtype=np.int32)])
    order = np.argsort(dst, kind="stable")
    src, dst = src[order].astype(np.int64), dst[order].astype(np.int64)
    core_of = dst // cfg.V
    bounds = np.searchsorted(core_of, np.arange(cfg.NCORES + 1))

    # ---- pass A: per-core group counts + block-balancing permutation ----
    # used_chunks is ceil(max-over-cores blk_tot / 16), so balancing group
    # counts across blocks (via a node permutation) trims gather chunks.
    perms, invs, ngrps, eds, ess = [], [], [], [], []
    caps = [128] * (cfg.NB - 1) + [cfg.V - 128 * (cfg.NB - 1)]
    for k in range(cfg.NCORES):
        lo, hi = bounds[k], bounds[k + 1]
        es = src[lo:hi]
        ed = dst[lo:hi] - k * cfg.V               # local dst, sorted
        deg = np.bincount(ed, minlength=cfg.V).astype(np.int64)
        # every node has >=1 incoming (the appended self loop)
        assert deg.min() >= 1
        degr = deg - 1
        ngrp = np.maximum(1, (degr + 6) // 7)
        # greedy LPT: heaviest nodes first into the lightest open block
        order_n = np.argsort(-ngrp, kind="stable")
        counts = [0] * cfg.NB
        members = [[] for _ in range(cfg.NB)]
        heap = [(0, b) for b in range(cfg.NB)]
        heapq.heapify(heap)
        for n in order_n:
            while True:
                s, b = heapq.heappop(heap)
                if counts[b] < caps[b]:
                    break
            members[b].append(n)
            counts[b] += 1
            if counts[b] < caps[b]:
                heapq.heappush(heap, (s + int(ngrp[n]), b))
        perm = np.concatenate([np.array(m, dtype=np.int64)
                               for m in members])      # new -> old
        inv = np.empty(cfg.V, np.int64)
        inv[perm] = np.arange(cfg.V)                   # old -> new
        perms.append(perm)
        invs.append(inv)
        ngrps.append(ngrp)
        eds.append(ed)
        ess.append(es)
    # old global id -> permuted table row id. Table layout is
    # [half0 of every core | half1 of every core] so the table AllGather can
    # be split into two half-shard collectives (see allgather_half).
    Vh = cfg.V // 2

    def _t_row(k, i):
        if not SPLIT_AG:
            return k * cfg.V + i
        return (i >= Vh) * (cfg.NCORES * Vh) + k * Vh + (i % Vh)

    g2t = np.concatenate([_t_row(k, invs[k]) for k in range(cfg.NCORES)])

    out = []
    for k in range(cfg.NCORES):
        perm, inv = perms[k], invs[k]
        # re-sort edges by permuted dst (stable keeps the appended self
        # edge last within each run)
        ed_new = inv[eds[k]]
        order2 = np.argsort(ed_new, kind="stable")
        es = g2t[ess[k][order2]]                  # src as table row ids
        ed = ed_new[order2]
        deg = np.bincount(ed, minlength=cfg.V).astype(np.int64)
        estart = np.zeros(cfg.V + 1, np.int64)
        np.cumsum(deg, out=estart[1:])
        degr = deg - 1
        ngrp = np.maximum(1, (degr + 6) // 7)

        # empty slots gather row 0 (harmless) and carry valid=0, so no
        # sentinel table row is needed (keeps the table single-writer =
        # AllGather only, required for Shared DRAM)
        src_slot = np.zeros((128, cfg.NCHUNK), np.int32)
        valid = np.zeros((128, cfg.NCHUNK), np.float32)
        glab = np.full(cfg.G, 1e9, np.float32)         # in-block node label

        nodes = np.arange(cfg.V, dtype=np.int64)
        blk = nodes // 128
        cumg = np.cumsum(ngrp)
        blk_start_node = blk * 128
        cumg_before_block = np.where(blk_start_node > 0,
                                     cumg[blk_start_node - 1], 0)
        gbase_n = (cumg - ngrp) - cumg_before_block
        blk_tot = np.zeros(cfg.NB, np.int64)
        np.add.at(blk_tot, blk, ngrp)
        assert blk_tot.max() <= cfg.GB, (
            f"core {k}: max groups/block {blk_tot.max()} > {cfg.GB}")
        # groups: labels + slot-0 self rows
        grp_node = np.repeat(nodes, ngrp)               # local node per group
        within = np.arange(len(grp_node), dtype=np.int64) - \
            np.repeat(cumg - ngrp, ngrp)                # 0..ngrp-1
        grel = gbase_n[grp_node] + within               # in-block group idx
        g_global = blk[grp_node] * cfg.GB + grel
        glab[g_global] = (grp_node % 128).astype(np.float32)
        lab_g = grel % 16
        chunk_g = blk[grp_node] * cfg.CPB + grel // 16
        src_slot[lab_g, chunk_g] = _t_row(k, grp_node).astype(np.int32)
        valid[lab_g[within == 0], chunk_g[within == 0]] = 1.0
        # non-self edges -> slots 1..7
        n_e = ed
        j_in = np.arange(len(ed), dtype=np.int64) - estart[n_e]
        keep = j_in < degr[n_e]     # drops the appended self edge (last)
        n_k = n_e[keep]
        j_k = j_in[keep]
        grel_e = gbase_n[n_k] + j_k // 7
        lab = grel_e % 16
        c = grel_e // 16
        p = lab + 16 * (1 + j_k % 7)
        chunk = blk[n_k] * cfg.CPB + c
        src_slot[p, chunk] = es[keep].astype(np.int32)
        valid[p, chunk] = 1.0
        # [p, T] layouts for the device
        glab_pt = glab.reshape(cfg.NT, 128).T.astype(np.float32).copy()
        out.append({
            "src_slot": src_slot,
            "glabel": glab_pt,
            "valid": valid.astype(BF16),
            "blk_tot": blk_tot.copy(),
            "perm": perm,
        })
    return out


def _used_chunks(cfg, pre):
    """Per-block chunk count actually carrying edges, maxed over cores (the
    NEFF is shared), rounded up to whole 16-group chunks."""
    mx = np.maximum.reduce([p["blk_tot"] for p in pre])
    return tuple(int(x) for x in np.minimum((mx + 15) // 16, cfg.CPB))


# ------------------------------------------------------------ bass build ---
def build_program(cfg, used_chunks=None):
    import concourse.bass as bass
    import concourse.bacc as bacc
    import concourse.mybir as mybir
    import concourse.tile as tile
    from concourse.bass import IndirectOffsetOnAxis, ds

    dt = mybir.dt
    F32, BF, I32 = dt.float32, dt.bfloat16, dt.int32
    AF = mybir.ActivationFunctionType
    OP = mybir.AluOpType
    HID, ROW, RHSW, NCLS = cfg.HID, cfg.ROW, cfg.RHS_W, cfg.NCLS

    if used_chunks is None:
        used_chunks = (cfg.CPB,) * cfg.NB
    nc = bacc.Bacc("TRN2", target_bir_lowering=False, debug=False,
                   enable_asserts=False, num_devices=cfg.NCORES)

    # ---- I/O ----
    xT = nc.dram_tensor("xT", [cfg.N_IN, cfg.VPAD], BF, kind="ExternalInput")
    W1 = nc.dram_tensor("W1", [cfg.N_IN, HID], F32, kind="ExternalInput")
    W2 = nc.dram_tensor("W2", [HID, HID], F32, kind="ExternalInput")
    Wo = nc.dram_tensor("Wo", [HID, NCLS], F32, kind="ExternalInput")
    a1s = nc.dram_tensor("a1s", [HID], F32, kind="ExternalInput")
    a1d = nc.dram_tensor("a1d", [HID], F32, kind="ExternalInput")
    a2s = nc.dram_tensor("a2s", [HID], F32, kind="ExternalInput")
    a2d = nc.dram_tensor("a2d", [HID], F32, kind="ExternalInput")
    b1 = nc.dram_tensor("b1", [HID], F32, kind="ExternalInput")
    b2 = nc.dram_tensor("b2", [HID], F32, kind="ExternalInput")
    bo = nc.dram_tensor("bo", [NCLS], F32, kind="ExternalInput")
    src_slot = nc.dram_tensor("src_slot", [128, cfg.NCHUNK], I32,
                              kind="ExternalInput")
    glab_in = nc.dram_tensor("glabel", [128, cfg.NT], F32,
                             kind="ExternalInput")
    valid_in = nc.dram_tensor("valid", [128, cfg.NCHUNK], BF,
                              kind="ExternalInput")
    out_t = nc.dram_tensor("out", [cfg.VPAD, NCLS], F32,
                           kind="ExternalOutput")

    # ---- inline constants ----
    ident_d = nc.inline_tensor(np.eye(128, dtype=np.float32), "ident")
    mask_np = (np.arange(128)[:, None] % 16 == np.arange(16)[None, :])
    mask_d = nc.inline_tensor(mask_np.astype(BF16), "mask16")
    # transposed 16-label mask: [16, 128], mask16T[l, p] = (p % 16 == l)
    mask_t_np = (np.arange(128)[None, :] % 16 == np.arange(16)[:, None])
    mask_t_d = nc.inline_tensor(mask_t_np.astype(BF16), "mask16T")
    iota_d = nc.inline_tensor(
        np.tile(np.arange(128, dtype=np.float32), (128, 1)), "iota2d")

    groups = [list(range(cfg.NCORES))]

    with tile.TileContext(nc, num_cores=cfg.NCORES) as tc:
        with (
            tc.tile_pool(name="const", bufs=1) as cp,
            tc.tile_pool(name="work", bufs=4) as wp,
            tc.tile_pool(name="gsum", bufs=6) as sp,
            tc.tile_pool(name="psum", bufs=2, space="PSUM") as pp,
            tc.tile_pool(name="dram", bufs=1, space="DRAM") as dp,
        ):
            # ======== constants to SBUF ========
            ident = cp.tile([128, 128], F32, tag="ident")
            nc.sync.dma_start(ident[:], ident_d[:, :])
            mask16 = cp.tile([128, 16], BF, tag="mask16")
            nc.sync.dma_start(mask16[:], mask_d[:, :])
            mask16t = cp.tile([16, 128], BF, tag="mask16t")
            nc.sync.dma_start(mask16t[:], mask_t_d[:, :])
            iota2 = cp.tile([128, 128], F32, tag="iota2")
            nc.sync.dma_start(iota2[:], iota_d[:, :])
            srcsb = cp.tile([128, cfg.NCHUNK], I32, tag="srcsb")
            nc.sync.dma_start(srcsb[:], src_slot[:, :])
            glabsb = cp.tile([128, cfg.NT], F32, tag="glabsb")
            nc.sync.dma_start(glabsb[:], glab_in[:, :])
            validsb = cp.tile([128, cfg.NCHUNK], BF, tag="validsb")
            nc.sync.dma_start(validsb[:], valid_in[:, :])
            wosb = cp.tile([128, NCLS], F32, tag="wosb")
            nc.sync.dma_start(wosb[:], Wo[:, :])
            b1r = cp.tile([128, HID], F32, tag="b1r")
            nc.sync.dma_start(b1r[:], b1[None, :].to_broadcast([128, HID]))
            b2r = cp.tile([128, HID], F32, tag="b2r")
            nc.sync.dma_start(b2r[:], b2[None, :].to_broadcast([128, HID]))
            bor = cp.tile([128, NCLS], F32, tag="bor")
            nc.sync.dma_start(bor[:], bo[None, :].to_broadcast([128, NCLS]))

            def make_rhs(Wd, asd_s, asd_d, nchunks, tagbase, out_bf=False):
                """rhs tiles [128, HID+2] = [W chunk | W@a_src | W@a_dst]."""
                asd = cp.tile([128, 2], F32, tag=tagbase + "_asd")
                nc.sync.dma_start(asd[:, 0:1], asd_s[:, None])
                nc.sync.dma_start(asd[:, 1:2], asd_d[:, None])
                tiles = []
                for c in range(nchunks):
                    rt = cp.tile([128, HID + 2], F32, tag=f"{tagbase}_{c}")
                    nc.sync.dma_start(rt[:, 0:HID],
                                      Wd[c * 128:(c + 1) * 128, :])
                    tp = pp.tile([128, 128], F32, tag="mm")
                    nc.tensor.transpose(tp[:], rt[:, 0:HID], ident[:])
                    wt = wp.tile([128, 128], F32, tag="wt")
                    nc.scalar.activation(wt[:], tp[:], AF.Copy)
                    sp2 = pp.tile([128, 2], F32, tag="mm")
                    nc.tensor.matmul(sp2[:], lhsT=wt[:], rhs=asd[:],
                                     start=True, stop=True)
                    nc.vector.tensor_copy(rt[:, HID:HID + 2], sp2[:])
                    if out_bf:
                        rtb = cp.tile([128, HID + 2], BF, tag=f"{tagbase}b{c}")
                        nc.vector.tensor_copy(rtb[:], rt[:])
                        tiles.append(rtb)
                    else:
                        tiles.append(rt)
                return tiles

            # layer-1 projection in bf16 (x is ~N(0,1); f32 PSUM accumulate
            # keeps the error well inside budget) -> half the xT DMA and 2x
            # the prep matmul rate
            rhs1 = make_rhs(W1, a1s, a1d, cfg.N_IN // 128, "rhs1",
                            out_bf=True)
            rhs2 = make_rhs(W2, a2s, a2d, 1, "rhs2")

            # persistent selector tiles: per chunk-pair layout is
            # [realA(16) | zeros(16) | realB(16)] so that the [*,32] lhsT
            # slice of either chunk has true zeros in its other half.
            # memset once; per-piece builds only touch the real columns.
            SELW = 48 * (cfg.PIECE_BLOCKS * cfg.CPB // 2)
            GTW = cfg.ROW * cfg.PIECE_BLOCKS * cfg.CPB
            # zero-fill via broadcast DMA (keeps GpSimd free for the
            # serialized indirect gathers, its real bottleneck)
            zrow_d = nc.inline_tensor(
                np.zeros((1, max(SELW, GTW)), dtype=BF16), "zrow")
            selA = cp.tile([128, SELW], BF, tag="selA")
            selB = cp.tile([128, SELW], BF, tag="selB")
            nc.sync.dma_start(selA[:],
                              zrow_d[0:1, 0:SELW].to_broadcast([128, SELW]))
            nc.sync.dma_start(selB[:],
                              zrow_d[0:1, 0:SELW].to_broadcast([128, SELW]))
            gtA = cp.tile([128, GTW], BF, tag="gtA")
            gtB = cp.tile([128, GTW], BF, tag="gtB")
            nc.sync.dma_start(gtA[:],
                              zrow_d[0:1, 0:GTW].to_broadcast([128, GTW]))
            nc.sync.dma_start(gtB[:],
                              zrow_d[0:1, 0:GTW].to_broadcast([128, GTW]))
            ones1 = cp.tile([128, 1], F32, tag="ones1")
            nc.gpsimd.memset(ones1[:], 1.0)

            # DRAM scratch (tables are AllGather outputs -> Shared address
            # space, else the collective bounces through an extra copy)
            tables = [dp.tile([cfg.N, ROW], BF, name=f"table{i}",
                              tag=f"table{i}", addr_space="Shared")
                      for i in range(2)]
            shards = [dp.tile([cfg.V, ROW], BF, name=f"shard{i}",
                              tag=f"shard{i}") for i in range(2)]

            # ======== per-layer table prep ========
            def build_rows(ppre, b, shard):
                """ppre: psum [128, HID+2] = [h' | s | d] for block b.

                Row layout: [h | one | s_hi d_hi | s_lo d_lo] so both hi
                halves are one scalar copy and both lo halves one vector
                subtract (halves the per-block dependency chain)."""
                rows = wp.tile([128, ROW], BF, tag="rows")
                nc.vector.tensor_copy(rows[:, 0:HID], ppre[:, 0:HID])
                nc.scalar.activation(rows[:, HID:HID + 1], ones1[:], AF.Copy)
                nc.scalar.activation(rows[:, HID + 1:HID + 3],
                                     ppre[:, HID:HID + 2], AF.Copy)
                nc.vector.tensor_tensor(rows[:, HID + 3:HID + 5],
                                        ppre[:, HID:HID + 2],
                                        rows[:, HID + 1:HID + 3],
                                        op=OP.subtract)
                nrow = min(128, cfg.V - b * 128)
                nc.scalar.dma_start(shard[b * 128:b * 128 + nrow, :],
                                    rows[0:nrow, :])

            # split allgathers: table layout is [half0 of every core |
            # half1 of every core] so each half-shard allgather can fire as
            # soon as its rows are written, hiding the collective behind
            # prep (layer 1) / main-loop compute (layer 2).
            Vh = cfg.V // 2
            AG_SPLIT_BLOCK = Vh // 128          # shard block covering row Vh

            def allgather_half(li, h):
                if not SPLIT_AG:
                    if h == 1:
                        nc.gpsimd.collective_compute(
                            "AllGather", mybir.AluOpType.bypass,
                            replica_groups=groups,
                            ins=[shards[li][:, :].opt()],
                            outs=[tables[li][0:cfg.N, :].opt()],
                        )
                    return
                lo = h * Vh
                out_lo = h * cfg.NCORES * Vh
                nc.gpsimd.collective_compute(
                    "AllGather", mybir.AluOpType.bypass,
                    replica_groups=groups,
                    ins=[shards[li][lo:lo + Vh, :].opt()],
                    outs=[tables[li][out_lo:out_lo + cfg.NCORES * Vh,
                                     :].opt()],
                )

            def prep_layer1():
                NCX = cfg.N_IN // 128
                for b in range(cfg.NB):
                    ppre = pp.tile([128, HID + 2], F32, tag="mm")
                    # both 128-row xT chunks of this block in ONE DMA
                    # (fewer serial DMA-issue slots on the Sync sequencer)
                    xt2 = wp.tile([128, NCX * 128], BF, tag="xt")
                    src = xT[:, b * 128:(b + 1) * 128].rearrange(
                        "(c p) w -> p c w", p=128)
                    nc.sync.dma_start(
                        xt2[:].rearrange("p (c w) -> p c w", c=NCX), src)
                    for c in range(NCX):
                        nc.tensor.matmul(ppre[:],
                                         lhsT=xt2[:, c * 128:(c + 1) * 128],
                                         rhs=rhs1[c][:],
                                         start=(c == 0),
                                         stop=(c == NCX - 1))
                    build_rows(ppre, b, shards[0])
                    if b == AG_SPLIT_BLOCK:
                        allgather_half(0, 0)

            # ======== main per-layer loop ========
            def main_layer(li, epilogue, mid_cb=None):
                table = tables[li]
                PB = cfg.PIECE_BLOCKS
                for pi, (b0, nb) in enumerate(cfg.pieces):
                    NCh = nb * cfg.CPB
                    # feature gather. SWDGE cost is ~994ns fixed per
                    # instruction + 0.34ns/descriptor, so batching chunks
                    # into one indirect DMA cuts GpSimd issue time. Unused
                    # tail chunks gather row 0: zero-weighted junk.
                    gt = gtA if pi % 2 == 0 else gtB
                    if GATHER_MODE == "hwloop":
                        # hardware loop: one decoded indirect-DMA replayed
                        # NCh times with register-stepped offsets (cuts the
                        # per-instruction sequencer fetch/dispatch cost)
                        with tc.For_i(0, NCh) as j:
                            nc.gpsimd.indirect_dma_start(
                                out=gt[:, ds(j * ROW, ROW)], out_offset=None,
                                in_=table[:, :],
                                in_offset=IndirectOffsetOnAxis(
                                    ap=srcsb[:, ds(b0 * cfg.CPB + j, 1)],
                                    axis=0))
                    elif GATHER_MODE == "piece":
                        off = srcsb[:, b0 * cfg.CPB:(b0 + nb) * cfg.CPB]
                        dstv = gt[:, 0:ROW * NCh].rearrange(
                            "p (j r) -> p j r", r=ROW)
                        nc.gpsimd.indirect_dma_start(
                            out=dstv, out_offset=None,
                            in_=table[:, :],
                            in_offset=IndirectOffsetOnAxis(ap=off, axis=0))
                    elif GATHER_MODE == "block":
                        for bb in range(nb):
                            j0 = bb * cfg.CPB
                            off = srcsb[:, (b0 + bb) * cfg.CPB:
                                        (b0 + bb + 1) * cfg.CPB]
                            dstv = gt[:, ROW * j0:ROW * (j0 + cfg.CPB)] \
                                .rearrange("p (j r) -> p j r", r=ROW)
                            nc.gpsimd.indirect_dma_start(
                                out=dstv, out_offset=None,
                                in_=table[:, :],
                                in_offset=IndirectOffsetOnAxis(ap=off, axis=0))
                    else:  # per-chunk (original)
                        for j in range(NCh):
                            if (j % cfg.CPB) >= used_chunks[b0 + j // cfg.CPB]:
                                continue
                            nc.gpsimd.indirect_dma_start(
                                out=gt[:, ROW * j:ROW * (j + 1)],
                                out_offset=None,
                                in_=table[:, :],
                                in_offset=IndirectOffsetOnAxis(
                                    ap=srcsb[:, b0 * cfg.CPB + j:
                                             b0 * cfg.CPB + j + 1],
                                    axis=0))
                    gv = gt[:, 0:ROW * NCh].rearrange(
                        "p (j c) -> p j c", c=ROW)
                    # d_dst per slot: broadcast slot-0 rows' d across the 8
                    # slot positions with one 16-contraction matmul pair
                    pd = pp.tile([128, PB * cfg.CPB], F32, tag="pd")
                    gvt = gt[0:16, 0:ROW * NCh].rearrange(
                        "p (j c) -> p j c", c=ROW)
                    nc.tensor.matmul(
                        pd[:, 0:NCh], lhsT=mask16t[:],
                        rhs=gvt[:, :, HID + 2:HID + 3].rearrange(
                            "p j c -> p (j c)"),
                        start=True, stop=False)
                    nc.tensor.matmul(
                        pd[:, 0:NCh], lhsT=mask16t[:],
                        rhs=gvt[:, :, HID + 4:HID + 5].rearrange(
                            "p j c -> p (j c)"),
                        start=False, stop=True)
                    # phase A: ex = exp(leaky_relu(s_src + d_dst)) * valid
                    sf = wp.tile([128, PB * cfg.CPB], F32, tag="sf")
                    sfv = sf[:, 0:NCh].rearrange("p j -> p j ()")
                    nc.vector.tensor_tensor(
                        sfv, gv[:, :, HID + 1:HID + 2],
                        gv[:, :, HID + 3:HID + 4], op=OP.add)
                    ep = wp.tile([128, PB * cfg.CPB], F32, tag="ep")
                    nc.vector.tensor_tensor(ep[:, 0:NCh], sf[:, 0:NCh],
                                            pd[:, 0:NCh], op=OP.add)
                    es = wp.tile([128, PB * cfg.CPB], F32, tag="es")
                    nc.vector.tensor_scalar_mul(es[:, 0:NCh], ep[:, 0:NCh],
                                                0.2)
                    el = wp.tile([128, PB * cfg.CPB], F32, tag="el")
                    nc.vector.tensor_tensor(el[:, 0:NCh], ep[:, 0:NCh],
                                            es[:, 0:NCh], op=OP.max)
                    exf = wp.tile([128, PB * cfg.CPB], F32, tag="exf")
                    nc.scalar.activation(exf[:, 0:NCh], el[:, 0:NCh], AF.Exp)
                    exb = wp.tile([128, PB * cfg.CPB], BF, tag="exb")
                    nc.vector.tensor_tensor(
                        exb[:, 0:NCh], exf[:, 0:NCh],
                        validsb[:, b0 * cfg.CPB:b0 * cfg.CPB + NCh],
                        op=OP.mult)
                    # selector build: even chunks -> cols [48a, 48a+16),
                    # odd chunks -> cols [48a+32, 48a+48)
                    sel1 = selA if (b0 // cfg.PIECE_BLOCKS) % 2 == 0 else selB
                    npair = NCh // 2
                    exv = exb[:, 0:NCh].rearrange("p (a u) -> p a u", u=2)
                    maskv = mask16[:].rearrange("p l -> p () l") \
                        .to_broadcast([128, npair, 16])
                    selv = sel1[:, 0:48 * npair].rearrange(
                        "p (a w) -> p a w", w=48)
                    nc.vector.tensor_tensor(
                        selv[:, :, 0:16],
                        exv[:, :, 0:1].to_broadcast([128, npair, 16]),
                        maskv, op=OP.mult)
                    nc.vector.tensor_tensor(
                        selv[:, :, 32:48],
                        exv[:, :, 1:2].to_broadcast([128, npair, 16]),
                        maskv, op=OP.mult)
                    # level 1 + level 2
                    for bb in range(nb):
                        b = b0 + bb
                        pl2 = pp.tile([128, RHSW], F32, tag="l2")
                        for t in range(cfg.TPB):
                            pl1 = pp.tile([128, RHSW], F32, tag="l1")
                            for al in range(4):
                                for u in range(2):
                                    j = bb * cfg.CPB + t * 8 + 2 * al + u
                                    A = j // 2
                                    nc.tensor.matmul(
                                        pl1[32 * al:32 * al + 32, :],
                                        lhsT=sel1[:, 48 * A + 16 * u:
                                                  48 * A + 16 * u + 32],
                                        rhs=gt[:, ROW * j:ROW * j + RHSW],
                                        start=(u == 0), stop=(u == 1),
                                        tile_position=(0, 32 * al))
                            gs = sp.tile([128, RHSW], F32, tag="gsum")
                            nc.scalar.activation(gs[:], pl1[:], AF.Copy)
                            T = b * cfg.TPB + t
                            l2s = wp.tile([128, 128], F32, tag="l2s")
                            nc.vector.tensor_tensor(
                                l2s[:],
                                glabsb[:, T:T + 1].to_broadcast([128, 128]),
                                iota2[:], op=OP.is_equal)
                            nc.tensor.matmul(pl2[:], lhsT=l2s[:], rhs=gs[:],
                                             start=(t == 0),
                                             stop=(t == cfg.TPB - 1))
                        epilogue(b, pl2)
                    if mid_cb is not None:
                        mid_cb(pi)

            def epi_norm(pl2, brow):
                den = wp.tile([128, 1], F32, tag="den")
                nc.vector.tensor_scalar_max(den[:], pl2[:, HID:HID + 1],
                                            1e-30)
                rec = wp.tile([128, 1], F32, tag="rec")
                nc.vector.reciprocal(rec[:], den[:])
                hb = wp.tile([128, HID], F32, tag="hb")
                nc.vector.tensor_scalar_mul(hb[:], pl2[:, 0:HID],
                                            rec[:, 0:1])
                nc.vector.tensor_tensor(hb[:], hb[:], brow[:], op=OP.add)
                return hb

            def epilogue1(b, pl2):
                # fused layer-2 prep: h2 -> [h'|s|d] rows for block b, so
                # only the AllGather sits between the two main loops
                hb = epi_norm(pl2, b1r)
                h2 = wp.tile([128, HID], F32, tag="h2")
                nc.scalar.activation(h2[:], hb[:], AF.Relu)
                tp = pp.tile([128, 128], F32, tag="mm")
                nc.tensor.transpose(tp[:], h2[:], ident[:])
                h2T = wp.tile([128, 128], F32, tag="h2T")
                nc.scalar.activation(h2T[:], tp[:], AF.Copy)
                ppre = pp.tile([128, HID + 2], F32, tag="mm")
                nc.tensor.matmul(ppre[:], lhsT=h2T[:], rhs=rhs2[0][:],
                                 start=True, stop=True)
                build_rows(ppre, b, shards[1])

            def epilogue2(b, pl2):
                hb = epi_norm(pl2, b2r)
                h3 = wp.tile([128, HID], F32, tag="h3")
                nc.scalar.activation(h3[:], hb[:], AF.Relu)
                tp = pp.tile([128, 128], F32, tag="mm")
                nc.tensor.transpose(tp[:], h3[:], ident[:])
                h3T = wp.tile([128, 128], F32, tag="h3T")
                nc.scalar.activation(h3T[:], tp[:], AF.Copy)
                po = pp.tile([128, NCLS], F32, tag="mm")
                nc.tensor.matmul(po[:], lhsT=h3T[:], rhs=wosb[:],
                                 start=True, stop=True)
                lg = wp.tile([128, NCLS], F32, tag="lg")
                nc.vector.tensor_tensor(lg[:], po[:], bor[:], op=OP.add)
                mx = wp.tile([128, 1], F32, tag="mx")
                nc.vector.tensor_reduce(mx[:], lg[:],
                                        axis=mybir.AxisListType.X, op=OP.max)
                lgs = wp.tile([128, NCLS], F32, tag="lgs")
                nc.vector.tensor_scalar(lgs[:], lg[:], mx[:, 0:1], None,
                                        op0=OP.subtract)
                pe = wp.tile([128, NCLS], F32, tag="pe")
                rs = wp.tile([128, 1], F32, tag="rs")
                nc.scalar.activation(pe[:], lgs[:], AF.Exp,
                                     accum_out=rs[:, 0:1])
                rr = wp.tile([128, 1], F32, tag="rr")
                nc.vector.reciprocal(rr[:], rs[:])
                ot = wp.tile([128, NCLS], F32, tag="ot")
                nc.vector.tensor_scalar_mul(ot[:], pe[:], rr[:, 0:1])
                nc.sync.dma_start(out_t[b * 128:(b + 1) * 128, :], ot[:])

            # ======== schedule ========
            # ag half-0s fire early (inside prep_layer1 / mid main-loop) so
            # only the half-1 collectives sit on the critical path.
            AG1_PIECE = max(0, min(len(cfg.pieces) - 2,
                                   (AG_SPLIT_BLOCK // cfg.PIECE_BLOCKS) + 3))

            def mid1(pi):
                if pi == AG1_PIECE:
                    allgather_half(1, 0)

            prep_layer1()
            allgather_half(0, 1)
            main_layer(0, epilogue1, mid_cb=mid1)
            allgather_half(1, 1)
            main_layer(1, epilogue2)

    nc.compile()
    return nc


# ------------------------------------------------------------------ run ---
_PROG_CACHE = {}


def _get_program(cfg, used_chunks=None):
    key = (cfg.N, cfg.E, cfg.NCORES, used_chunks)
    if key not in _PROG_CACHE:
        _PROG_CACHE[key] = build_program(cfg, used_chunks)
    return _PROG_CACHE[key]


_PRE_CACHE = {}


def run(cfg, inputs, trace=False, tmpdir=None):
    from concourse.bass_utils import run_bass_kernel_spmd

    ei = np.asarray(inputs["edge_index"])
    pkey = (cfg.N, cfg.E, int(ei[0, :16].sum()), int(ei[1, -16:].sum()))
    if pkey not in _PRE_CACHE:
        _PRE_CACHE[pkey] = preprocess(cfg, ei)
    pre = _PRE_CACHE[pkey]
    nc = _get_program(cfg, _used_chunks(cfg, pre))
    in_maps = _make_in_maps(cfg, inputs, pre)
    kw = {}
    if trace:
        kw = dict(trace=True, tmpdir=tmpdir)
    res = run_bass_kernel_spmd(nc, in_maps, core_ids=list(range(cfg.NCORES)),
                               **kw)
    outs = []
    for k in range(cfg.NCORES):
        o = res.results[k]["out"][:cfg.V]
        oo = np.empty_like(o)
        oo[pre[k]["perm"]] = o                   # undo block-balancing perm
        outs.append(oo)
    full = np.concatenate(outs, axis=0).astype(np.float32)
    return full, res


def timed_run(cfg, inputs, iters=3):
    """Time device execution with device-resident inputs (axon transfer
    excluded). Returns (best_seconds, outputs_of_last_iter)."""
    import time

    import jax
    from jax.sharding import Mesh, PartitionSpec
    from jax.experimental.shard_map import shard_map
    import concourse.mybir as mybir
    from concourse import bass2jax
    from concourse.bass2jax import _bass_exec_p, partition_id_tensor

    bass2jax.install_neuronx_cc_hook()
    ei = np.asarray(inputs["edge_index"])
    pkey = (cfg.N, cfg.E, int(ei[0, :16].sum()), int(ei[1, -16:].sum()))
    if pkey not in _PRE_CACHE:
        _PRE_CACHE[pkey] = preprocess(cfg, ei)
    pre = _PRE_CACHE[pkey]
    nc = _get_program(cfg, _used_chunks(cfg, pre))
    in_maps = _make_in_maps(cfg, inputs, pre)

    partition_name = (nc.partition_id_tensor.name
                      if nc.partition_id_tensor else None)
    in_names, out_names, out_avals, zero_outs = [], [], [], []
    for alloc in nc.m.functions[0].allocations:
        if not isinstance(alloc, mybir.MemoryLocationSet):
            continue
        name = alloc.memorylocations[0].name
        if alloc.kind == "ExternalInput":
            if name != partition_name:
                in_names.append(name)
        elif alloc.kind == "ExternalOutput":
            out_names.append(name)
            shape = tuple(alloc.tensor_shape)
            dtype = mybir.dt.np(alloc.dtype)
            out_avals.append(jax.core.ShapedArray(shape, dtype))
            zero_outs.append(np.zeros(shape, dtype))
    n_params = len(in_names)
    n_outs = len(out_avals)
    all_in_names = list(in_names) + list(out_names)
    if partition_name is not None:
        all_in_names.append(partition_name)
    donate = tuple(range(n_params, n_params + n_outs))

    def _body(*args):
        operands = list(args)
        if partition_name is not None:
            operands.append(partition_id_tensor())
        outs = _bass_exec_p.bind(
            *operands, out_avals=tuple(out_avals),
            in_names=tuple(all_in_names), out_names=tuple(out_names),
            lowering_input_output_aliases=(),
            sim_require_finite=True, sim_require_nnan=True, nc=nc)
        return tuple(outs)

    devices = jax.devices()[:cfg.NCORES]
    mesh = Mesh(np.asarray(devices), ("core",))
    in_specs = (PartitionSpec("core"),) * (n_params + n_outs)
    out_specs = (PartitionSpec("core"),) * len(out_names)
    sharded = jax.jit(
        shard_map(_body, mesh=mesh, in_specs=in_specs, out_specs=out_specs,
                  check_rep=False),
        donate_argnums=donate, keep_unused=True)
    concat_in = [
        np.concatenate([np.asarray(in_maps[c][nm]) for c in range(cfg.NCORES)],
                       axis=0)
        for nm in in_names]
    sharding = jax.sharding.NamedSharding(mesh, PartitionSpec("core"))
    dev_in = [jax.device_put(a, sharding) for a in concat_in]
    times = []
    out_arrs = None
    for _ in range(iters):
        zo = [jax.device_put(
            np.zeros((cfg.NCORES * z.shape[0], *z.shape[1:]), z.dtype),
            sharding) for z in zero_outs]
        jax.block_until_ready(zo)
        t0 = time.time()
        out_arrs = sharded(*dev_in, *zo)
        jax.block_until_ready(out_arrs)
        times.append(time.time() - t0)
    oi = out_names.index("out")
    full = np.asarray(out_arrs[oi]).reshape(cfg.NCORES, cfg.VPAD, cfg.NCLS)
    outs = []
    for c in range(cfg.NCORES):
        o = full[c, :cfg.V]
        oo = np.empty_like(o)
        oo[pre[c]["perm"]] = o
        outs.append(oo)
    out = np.concatenate(outs, axis=0)
    return min(times), out.astype(np.float32)


def _make_in_maps(cfg, inputs, pre):
    x = np.asarray(inputs["x"], np.float32)
    common = {
        "W1": np.asarray(inputs["W1"], np.float32),
        "W2": np.asarray(inputs["W2"], np.float32),
        "Wo": np.asarray(inputs["Wo"], np.float32),
        "a1s": np.asarray(inputs["a1_src"], np.float32),
        "a1d": np.asarray(inputs["a1_dst"], np.float32),
        "a2s": np.asarray(inputs["a2_src"], np.float32),
        "a2d": np.asarray(inputs["a2_dst"], np.float32),
        "b1": np.asarray(inputs["b1"], np.float32),
        "b2": np.asarray(inputs["b2"], np.float32),
        "bo": np.asarray(inputs["bo"], np.float32),
    }
    in_maps = []
    for k in range(cfg.NCORES):
        xs = x[k * cfg.V:(k + 1) * cfg.V][pre[k]["perm"]]
        xT = np.zeros((cfg.N_IN, cfg.VPAD), BF16)
        xT[:, :cfg.V] = xs.T.astype(BF16)
        m = dict(common)
        m["xT"] = np.ascontiguousarray(xT)
        if GATHER_MODE == "block" and HW_IDX_ORDER:
            m["src_slot"] = _wrap_idx_for_hw(pre[k]["src_slot"], cfg.CPB)
        else:
            m["src_slot"] = pre[k]["src_slot"]
        m["glabel"] = pre[k]["glabel"]
        m["valid"] = pre[k]["valid"]
        in_maps.append(m)
    return in_maps


def kernel(**inputs):
    cfg = DEFAULT_CFG
    full, _ = run(cfg, inputs, trace=False)
    return full



# revision 45
# speedup vs baseline: 1.0183x; 1.0095x over previous
"""Trainium2 Bass kernel for a 2-layer GAT (heads=1) + linear head + softmax.

Strategy (8 NeuronCores, graph/data parallel):
  - Nodes sharded across cores (12500 dst nodes each); edges partitioned by
    destination node so segment softmax / scatter stay local to a core.
  - Per layer, each core computes projected features for its node shard:
    table row = [h' (HID, bf16) | 1.0 | s_hi | s_lo | d_hi | d_lo]
    (s = h' @ a_src and d = h' @ a_dst, each split into two bf16 halves for
    ~f32 precision), then an AllGather replicates the full node table to
    every core's DRAM (halo exchange).
  - Edges are laid out in "slots": 8 slots per group, 16 groups per
    128-slot chunk, 24 chunks per 128-node destination block (384 groups =
    3 "gsum tiles" per block, padded -> identical program on every core).
    Slot 0 of every group holds the destination node's own table row: it is
    both the self-loop edge (valid=1 in the first group) and the carrier of
    d_dst for the group (duplicate carriers are zero-weighted via a host
    "valid" mask folded into the selector).
  - Main loop per layer: one big indirect-DMA gather of table rows per
    slot-chunk; d_dst per slot via one tiny matmul (mask16T x slot0-rows)
    broadcasting d across the 8 slot positions; per-edge attention logits
    e = leaky_relu(s_src + d_dst) and ex = exp(e) (no max-shift needed;
    logits are bounded), then a two-level matmul segment-reduction:
      level 1: ex*valid-carrying selector (static 16-label mask) x gathered
               rows -> per-group partial [sum(ex*h) | sum(ex)]
      level 2: is_equal(group-label, node-iota) selector x group partials
               -> per-node [numerator | denominator] accumulated in PSUM.
    Epilogue divides by the denominator (softmax normalization), adds bias,
    applies relu; layer 2 additionally applies the output head + softmax.

Performance notes (measured on HW):
  - The kernel is bound by the per-instruction cost of INDIRECT1D on
    GpSimd: ~1.1us Q7 SWDGE descriptor-gen (994ns fixed + 0.34ns/desc)
    plus ~310ns dispatch, i.e. ~1.43us per 128-row chunk regardless of
    row bytes or attached waits (verified with a dependency-free
    microbenchmark: 1426ns/gather). 2x2151 used chunks -> ~6.1ms floor.
  - Batching more rows per instruction is impossible in this environment:
    multi-column offset APs are mis-lowered by walrus (wrong descriptor
    count/elem size; [1,K] and [128,J>1] layouts crash or corrupt), and
    the batched InstDMAGatherAnt/extended-Q7 instructions are excluded
    from the bedrock image (running_on_bedrock()==True).
  - DMA queues run at ~20% occupancy; Tensor/Vector/Scalar fit entirely
    inside the gather-issue shadow. Head (prep+allgather) ~0.4ms, tail
    ~26us after the last gather.
"""

import math
import sys

import numpy as np

if "/opt/trn_rl_repo" not in sys.path:
    sys.path.insert(0, "/opt/trn_rl_repo")

import ml_dtypes

BF16 = ml_dtypes.bfloat16


# ---------------------------------------------------------------- config ---
class Cfg:
    def __init__(self, N, E, n_in=256, hid=128, ncls=3, ncores=8,
                 piece_blocks=5, gb=384):
        self.N, self.E = N, E
        self.N_IN, self.HID, self.NCLS = n_in, hid, ncls
        self.NCORES = ncores
        assert N % ncores == 0
        self.V = N // ncores                      # real dst nodes per core
        self.NB = math.ceil(self.V / 128)         # node blocks per core
        self.VPAD = self.NB * 128
        self.GB = gb                              # groups per block (padded)
        assert gb % 16 == 0
        self.CPB = gb // 16                       # chunks per block
        self.G = self.NB * self.GB                # groups per core
        assert self.G % 128 == 0
        self.NT = self.G // 128                   # gsum tiles per core (NB*3)
        self.TPB = self.GB // 128                 # gsum tiles per block
        assert self.GB % 128 == 0
        self.NCHUNK = self.NB * self.CPB
        self.SLOTS = self.NCHUNK * 128
        self.SENT = N                             # sentinel table row index
        self.ROW = hid + 5            # h | one | s_hi | s_lo | d_hi | d_lo
        self.RHS_W = hid + 1                      # matmul rhs width (h | one)
        # pieces: (block_start, nblocks); keep the final piece at 1 block so
        # the serial tail (last gather -> compute -> allgather/output) is
        # as short as possible
        self.pieces = []
        b = 0
        while b < self.NB:
            nb = min(piece_blocks, self.NB - b)
            self.pieces.append((b, nb))
            b += nb
        if self.pieces and self.pieces[-1][1] > 1:
            b0, nb = self.pieces[-1]
            self.pieces[-1] = (b0, nb - 1)
            self.pieces.append((b0 + nb - 1, 1))
        self.PIECE_BLOCKS = piece_blocks


DEFAULT_CFG = Cfg(N=100000, E=1600000)

# indirect-gather batching granularity: "piece" | "block" | "chunk"
GATHER_MODE = "chunk"
# split each table AllGather into two half-shard collectives (hidden behind
# prep/compute). CoreSim's Shared-DRAM model insists on a single writer per
# tensor, so set False when running under CoreSim.
SPLIT_AG = False
# HW INDIRECT1D consumes the index buffer channel-wrapped (flat index k read
# from partition k%128, column k//128) while the destination AP iterates
# partition-slowest; CoreSim pairs both in AP order. Host-permute indices for
# HW; set False when running under CoreSim.
HW_IDX_ORDER = True


def _wrap_idx_for_hw(src_slot, J):
    """Permute each J-column section so the HW channel-wrapped index read
    matches the destination's AP iteration order (partition-major)."""
    out = np.empty_like(src_slot)
    P, NCH = src_slot.shape
    k = np.arange(P * J)
    for c0 in range(0, NCH, J):
        out[k % P, c0 + k // P] = src_slot[k // J, c0 + (k % J)]
    return out


# ---------------------------------------------------- host preprocessing ---
def preprocess(cfg, edge_index):
    """Partition edges by destination core and build per-core slot layout.

    Slot 0 (partition = group label) of every group carries the destination
    node's own row: the self-loop edge for the node's first group, a
    zero-weighted d-carrier duplicate for subsequent groups. The remaining
    7 slots per group hold the node's other incoming edges.

    Returns per-core dict of index tensors (identical shapes on every core
    so one NEFF serves all 8).
    """
    import heapq

    src = np.concatenate([edge_index[0], np.arange(cfg.N, dtype=np.int32)])
    dst = np.concatenate([edge_index[1], np.arange(cfg.N, dtype=np.int32)])
    order = np.argsort(dst, kind="stable")
    src, dst = src[order].astype(np.int64), dst[order].astype(np.int64)
    core_of = dst // cfg.V
    bounds = np.searchsorted(core_of, np.arange(cfg.NCORES + 1))

    # ---- pass A: per-core group counts + block-balancing permutation ----
    # used_chunks is ceil(max-over-cores blk_tot / 16), so balancing group
    # counts across blocks (via a node permutation) trims gather chunks.
    perms, invs, ngrps, eds, ess = [], [], [], [], []
    caps = [128] * (cfg.NB - 1) + [cfg.V - 128 * (cfg.NB - 1)]
    for k in range(cfg.NCORES):
        lo, hi = bounds[k], bounds[k + 1]
        es = src[lo:hi]
        ed = dst[lo:hi] - k * cfg.V               # local dst, sorted
        deg = np.bincount(ed, minlength=cfg.V).astype(np.int64)
        # every node has >=1 incoming (the appended self loop)
        assert deg.min() >= 1
        degr = deg - 1
        ngrp = np.maximum(1, (degr + 6) // 7)
        # greedy LPT: heaviest nodes first into the lightest open block
        order_n = np.argsort(-ngrp, kind="stable")
        counts = [0] * cfg.NB
        members = [[] for _ in range(cfg.NB)]
        heap = [(0, b) for b in range(cfg.NB)]
        heapq.heapify(heap)
        for n in order_n:
            while True:
                s, b = heapq.heappop(heap)
                if counts[b] < caps[b]:
                    break
            members[b].append(n)
            counts[b] += 1
            if counts[b] < caps[b]:
                heapq.heappush(heap, (s + int(ngrp[n]), b))
        perm = np.concatenate([np.array(m, dtype=np.int64)
                               for m in members])      # new -> old
        inv = np.empty(cfg.V, np.int64)
        inv[perm] = np.arange(cfg.V)                   # old -> new
        perms.append(perm)
        invs.append(inv)
        ngrps.append(ngrp)
        eds.append(ed)
        ess.append(es)
    # old global id -> permuted table row id. Table layout is
    # [half0 of every core | half1 of every core] so the table AllGather can
    # be split into two half-shard collectives (see allgather_half).
    Vh = cfg.V // 2

    def _t_row(k, i):
        if not SPLIT_AG:
            return k * cfg.V + i
        return (i >= Vh) * (cfg.NCORES * Vh) + k * Vh + (i % Vh)

    g2t = np.concatenate([_t_row(k, invs[k]) for k in range(cfg.NCORES)])

    out = []
    for k in range(cfg.NCORES):
        perm, inv = perms[k], invs[k]
        # re-sort edges by permuted dst (stable keeps the appended self
        # edge last within each run)
        ed_new = inv[eds[k]]
        order2 = np.argsort(ed_new, kind="stable")
        es = g2t[ess[k][order2]]                  # src as table row ids
        ed = ed_new[order2]
        deg = np.bincount(ed, minlength=cfg.V).astype(np.int64)
        estart = np.zeros(cfg.V + 1, np.int64)
        np.cumsum(deg, out=estart[1:])
        degr = deg - 1
        ngrp = np.maximum(1, (degr + 6) // 7)

        # empty slots gather row 0 (harmless) and carry valid=0, so no
        # sentinel table row is needed (keeps the table single-writer =
        # AllGather only, required for Shared DRAM)
        src_slot = np.zeros((128, cfg.NCHUNK), np.int32)
        valid = np.zeros((128, cfg.NCHUNK), np.float32)
        glab = np.full(cfg.G, 1e9, np.float32)         # in-block node label

        nodes = np.arange(cfg.V, dtype=np.int64)
        blk = nodes // 128
        cumg = np.cumsum(ngrp)
        blk_start_node = blk * 128
        cumg_before_block = np.where(blk_start_node > 0,
                                     cumg[blk_start_node - 1], 0)
        gbase_n = (cumg - ngrp) - cumg_before_block
        blk_tot = np.zeros(cfg.NB, np.int64)
        np.add.at(blk_tot, blk, ngrp)
        assert blk_tot.max() <= cfg.GB, (
            f"core {k}: max groups/block {blk_tot.max()} > {cfg.GB}")
        # groups: labels + slot-0 self rows
        grp_node = np.repeat(nodes, ngrp)               # local node per group
        within = np.arange(len(grp_node), dtype=np.int64) - \
            np.repeat(cumg - ngrp, ngrp)                # 0..ngrp-1
        grel = gbase_n[grp_node] + within               # in-block group idx
        g_global = blk[grp_node] * cfg.GB + grel
        glab[g_global] = (grp_node % 128).astype(np.float32)
        lab_g = grel % 16
        chunk_g = blk[grp_node] * cfg.CPB + grel // 16
        src_slot[lab_g, chunk_g] = _t_row(k, grp_node).astype(np.int32)
        valid[lab_g[within == 0], chunk_g[within == 0]] = 1.0
        # non-self edges -> slots 1..7
        n_e = ed
        j_in = np.arange(len(ed), dtype=np.int64) - estart[n_e]
        keep = j_in < degr[n_e]     # drops the appended self edge (last)
        n_k = n_e[keep]
        j_k = j_in[keep]
        grel_e = gbase_n[n_k] + j_k // 7
        lab = grel_e % 16
        c = grel_e // 16
        p = lab + 16 * (1 + j_k % 7)
        chunk = blk[n_k] * cfg.CPB + c
        src_slot[p, chunk] = es[keep].astype(np.int32)
        valid[p, chunk] = 1.0
        # [p, T] layouts for the device
        glab_pt = glab.reshape(cfg.NT, 128).T.astype(np.float32).copy()
        out.append({
            "src_slot": src_slot,
            "glabel": glab_pt,
            "valid": valid.astype(BF16),
            "blk_tot": blk_tot.copy(),
            "perm": perm,
        })
    return out


def _used_chunks(cfg, pre):
    """Per-block chunk count actually carrying edges, maxed over cores (the
    NEFF is shared), rounded up to whole 16-group chunks."""
    mx = np.maximum.reduce([p["blk_tot"] for p in pre])
    return tuple(int(x) for x in np.minimum((mx + 15) // 16, cfg.CPB))


# ------------------------------------------------------------ bass build ---
def build_program(cfg, used_chunks=None):
    import concourse.bass as bass
    import concourse.bacc as bacc
    import concourse.mybir as mybir
    import concourse.tile as tile
    from concourse.bass import IndirectOffsetOnAxis, ds

    dt = mybir.dt
    F32, BF, I32 = dt.float32, dt.bfloat16, dt.int32
    AF = mybir.ActivationFunctionType
    OP = mybir.AluOpType
    HID, ROW, RHSW, NCLS = cfg.HID, cfg.ROW, cfg.RHS_W, cfg.NCLS

    if used_chunks is None:
        used_chunks = (cfg.CPB,) * cfg.NB
    nc = bacc.Bacc("TRN2", target_bir_lowering=False, debug=False,
                   enable_asserts=False, num_devices=cfg.NCORES)

    # ---- I/O ----
    xT = nc.dram_tensor("xT", [cfg.N_IN, cfg.VPAD], BF, kind="ExternalInput")
    W1 = nc.dram_tensor("W1", [cfg.N_IN, HID], F32, kind="ExternalInput")
    W2 = nc.dram_tensor("W2", [HID, HID], F32, kind="ExternalInput")
    Wo = nc.dram_tensor("Wo", [HID, NCLS], F32, kind="ExternalInput")
    a1s = nc.dram_tensor("a1s", [HID], F32, kind="ExternalInput")
    a1d = nc.dram_tensor("a1d", [HID], F32, kind="ExternalInput")
    a2s = nc.dram_tensor("a2s", [HID], F32, kind="ExternalInput")
    a2d = nc.dram_tensor("a2d", [HID], F32, kind="ExternalInput")
    b1 = nc.dram_tensor("b1", [HID], F32, kind="ExternalInput")
    b2 = nc.dram_tensor("b2", [HID], F32, kind="ExternalInput")
    bo = nc.dram_tensor("bo", [NCLS], F32, kind="ExternalInput")
    src_slot = nc.dram_tensor("src_slot", [128, cfg.NCHUNK], I32,
                              kind="ExternalInput")
    glab_in = nc.dram_tensor("glabel", [128, cfg.NT], F32,
                             kind="ExternalInput")
    valid_in = nc.dram_tensor("valid", [128, cfg.NCHUNK], BF,
                              kind="ExternalInput")
    out_t = nc.dram_tensor("out", [cfg.VPAD, NCLS], F32,
                           kind="ExternalOutput")

    # ---- inline constants ----
    ident_d = nc.inline_tensor(np.eye(128, dtype=np.float32), "ident")
    mask_np = (np.arange(128)[:, None] % 16 == np.arange(16)[None, :])
    mask_d = nc.inline_tensor(mask_np.astype(BF16), "mask16")
    # transposed 16-label mask: [16, 128], mask16T[l, p] = (p % 16 == l)
    mask_t_np = (np.arange(128)[None, :] % 16 == np.arange(16)[:, None])
    mask_t_d = nc.inline_tensor(mask_t_np.astype(BF16), "mask16T")
    iota_d = nc.inline_tensor(
        np.tile(np.arange(128, dtype=np.float32), (128, 1)), "iota2d")

    groups = [list(range(cfg.NCORES))]

    with tile.TileContext(nc, num_cores=cfg.NCORES) as tc:
        with (
            tc.tile_pool(name="const", bufs=1) as cp,
            tc.tile_pool(name="work", bufs=4) as wp,
            tc.tile_pool(name="gsum", bufs=6) as sp,
            tc.tile_pool(name="psum", bufs=2, space="PSUM") as pp,
            tc.tile_pool(name="dram", bufs=1, space="DRAM") as dp,
        ):
            # ======== constants to SBUF ========
            ident = cp.tile([128, 128], F32, tag="ident")
            nc.sync.dma_start(ident[:], ident_d[:, :])
            mask16 = cp.tile([128, 16], BF, tag="mask16")
            nc.sync.dma_start(mask16[:], mask_d[:, :])
            mask16t = cp.tile([16, 128], BF, tag="mask16t")
            nc.sync.dma_start(mask16t[:], mask_t_d[:, :])
            iota2 = cp.tile([128, 128], F32, tag="iota2")
            nc.sync.dma_start(iota2[:], iota_d[:, :])
            srcsb = cp.tile([128, cfg.NCHUNK], I32, tag="srcsb")
            nc.sync.dma_start(srcsb[:], src_slot[:, :])
            glabsb = cp.tile([128, cfg.NT], F32, tag="glabsb")
            nc.sync.dma_start(glabsb[:], glab_in[:, :])
            validsb = cp.tile([128, cfg.NCHUNK], BF, tag="validsb")
            nc.sync.dma_start(validsb[:], valid_in[:, :])
            wosb = cp.tile([128, NCLS], F32, tag="wosb")
            nc.sync.dma_start(wosb[:], Wo[:, :])
            b1r = cp.tile([128, HID], F32, tag="b1r")
            nc.sync.dma_start(b1r[:], b1[None, :].to_broadcast([128, HID]))
            b2r = cp.tile([128, HID], F32, tag="b2r")
            nc.sync.dma_start(b2r[:], b2[None, :].to_broadcast([128, HID]))
            bor = cp.tile([128, NCLS], F32, tag="bor")
            nc.sync.dma_start(bor[:], bo[None, :].to_broadcast([128, NCLS]))

            def make_rhs(Wd, asd_s, asd_d, nchunks, tagbase, out_bf=False):
                """rhs tiles [128, HID+2] = [W chunk | W@a_src | W@a_dst]."""
                asd = cp.tile([128, 2], F32, tag=tagbase + "_asd")
                nc.sync.dma_start(asd[:, 0:1], asd_s[:, None])
                nc.sync.dma_start(asd[:, 1:2], asd_d[:, None])
                tiles = []
                for c in range(nchunks):
                    rt = cp.tile([128, HID + 2], F32, tag=f"{tagbase}_{c}")
                    nc.sync.dma_start(rt[:, 0:HID],
                                      Wd[c * 128:(c + 1) * 128, :])
                    tp = pp.tile([128, 128], F32, tag="mm")
                    nc.tensor.transpose(tp[:], rt[:, 0:HID], ident[:])
                    wt = wp.tile([128, 128], F32, tag="wt")
                    nc.scalar.activation(wt[:], tp[:], AF.Copy)
                    sp2 = pp.tile([128, 2], F32, tag="mm")
                    nc.tensor.matmul(sp2[:], lhsT=wt[:], rhs=asd[:],
                                     start=True, stop=True)
                    nc.vector.tensor_copy(rt[:, HID:HID + 2], sp2[:])
                    if out_bf:
                        rtb = cp.tile([128, HID + 2], BF, tag=f"{tagbase}b{c}")
                        nc.vector.tensor_copy(rtb[:], rt[:])
                        tiles.append(rtb)
                    else:
                        tiles.append(rt)
                return tiles

            # layer-1 projection in bf16 (x is ~N(0,1); f32 PSUM accumulate
            # keeps the error well inside budget) -> half the xT DMA and 2x
            # the prep matmul rate
            rhs1 = make_rhs(W1, a1s, a1d, cfg.N_IN // 128, "rhs1",
                            out_bf=True)
            rhs2 = make_rhs(W2, a2s, a2d, 1, "rhs2")

            # persistent selector tiles: per chunk-pair layout is
            # [realA(16) | zeros(16) | realB(16)] so that the [*,32] lhsT
            # slice of either chunk has true zeros in its other half.
            # memset once; per-piece builds only touch the real columns.
            SELW = 48 * (cfg.PIECE_BLOCKS * cfg.CPB // 2)
            GTW = cfg.ROW * cfg.PIECE_BLOCKS * cfg.CPB
            # zero-fill via broadcast DMA (keeps GpSimd free for the
            # serialized indirect gathers, its real bottleneck)
            zrow_d = nc.inline_tensor(
                np.zeros((1, max(SELW, GTW)), dtype=BF16), "zrow")
            selA = cp.tile([128, SELW], BF, tag="selA")
            selB = cp.tile([128, SELW], BF, tag="selB")
            nc.sync.dma_start(selA[:],
                              zrow_d[0:1, 0:SELW].to_broadcast([128, SELW]))
            nc.sync.dma_start(selB[:],
                              zrow_d[0:1, 0:SELW].to_broadcast([128, SELW]))
            gtA = cp.tile([128, GTW], BF, tag="gtA")
            gtB = cp.tile([128, GTW], BF, tag="gtB")
            nc.sync.dma_start(gtA[:],
                              zrow_d[0:1, 0:GTW].to_broadcast([128, GTW]))
            nc.sync.dma_start(gtB[:],
                              zrow_d[0:1, 0:GTW].to_broadcast([128, GTW]))
            ones1 = cp.tile([128, 1], F32, tag="ones1")
            nc.gpsimd.memset(ones1[:], 1.0)

            # DRAM scratch (tables are AllGather outputs -> Shared address
            # space, else the collective bounces through an extra copy)
            tables = [dp.tile([cfg.N, ROW], BF, name=f"table{i}",
                              tag=f"table{i}", addr_space="Shared")
                      for i in range(2)]
            shards = [dp.tile([cfg.V, ROW], BF, name=f"shard{i}",
                              tag=f"shard{i}") for i in range(2)]

            # ======== per-layer table prep ========
            def build_rows(ppre, b, shard, prep=False):
                """ppre: psum [128, HID+2] = [h' | s | d] for block b.

                Row layout: [h | one | s_hi d_hi | s_lo d_lo] so both hi
                halves are one scalar copy and both lo halves one vector
                subtract (halves the per-block dependency chain). During
                layer-1 prep the ones column goes to the idle GpSimd to
                unload the Scalar engine (the prep critical path); in the
                main loop GpSimd is the bottleneck so Scalar writes it."""
                rows = wp.tile([128, ROW], BF, tag="rows")
                nc.vector.tensor_copy(rows[:, 0:HID], ppre[:, 0:HID])
                if prep:
                    nc.gpsimd.memset(rows[:, HID:HID + 1], 1.0)
                else:
                    nc.scalar.activation(rows[:, HID:HID + 1], ones1[:],
                                         AF.Copy)
                nc.scalar.activation(rows[:, HID + 1:HID + 3],
                                     ppre[:, HID:HID + 2], AF.Copy)
                nc.vector.tensor_tensor(rows[:, HID + 3:HID + 5],
                                        ppre[:, HID:HID + 2],
                                        rows[:, HID + 1:HID + 3],
                                        op=OP.subtract)
                nrow = min(128, cfg.V - b * 128)
                nc.scalar.dma_start(shard[b * 128:b * 128 + nrow, :],
                                    rows[0:nrow, :])

            # split allgathers: table layout is [half0 of every core |
            # half1 of every core] so each half-shard allgather can fire as
            # soon as its rows are written, hiding the collective behind
            # prep (layer 1) / main-loop compute (layer 2).
            Vh = cfg.V // 2
            AG_SPLIT_BLOCK = Vh // 128          # shard block covering row Vh

            def allgather_half(li, h):
                if not SPLIT_AG:
                    if h == 1:
                        nc.gpsimd.collective_compute(
                            "AllGather", mybir.AluOpType.bypass,
                            replica_groups=groups,
                            ins=[shards[li][:, :].opt()],
                            outs=[tables[li][0:cfg.N, :].opt()],
                        )
                    return
                lo = h * Vh
                out_lo = h * cfg.NCORES * Vh
                nc.gpsimd.collective_compute(
                    "AllGather", mybir.AluOpType.bypass,
                    replica_groups=groups,
                    ins=[shards[li][lo:lo + Vh, :].opt()],
                    outs=[tables[li][out_lo:out_lo + cfg.NCORES * Vh,
                                     :].opt()],
                )

            def prep_layer1():
                NCX = cfg.N_IN // 128
                for b in range(cfg.NB):
                    ppre = pp.tile([128, HID + 2], F32, tag="mm")
                    # both 128-row xT chunks of this block in ONE DMA
                    # (fewer serial DMA-issue slots on the Sync sequencer)
                    xt2 = wp.tile([128, NCX * 128], BF, tag="xt")
                    src = xT[:, b * 128:(b + 1) * 128].rearrange(
                        "(c p) w -> p c w", p=128)
                    nc.sync.dma_start(
                        xt2[:].rearrange("p (c w) -> p c w", c=NCX), src)
                    for c in range(NCX):
                        nc.tensor.matmul(ppre[:],
                                         lhsT=xt2[:, c * 128:(c + 1) * 128],
                                         rhs=rhs1[c][:],
                                         start=(c == 0),
                                         stop=(c == NCX - 1))
                    build_rows(ppre, b, shards[0], prep=True)
                    if b == AG_SPLIT_BLOCK:
                        allgather_half(0, 0)

            # ======== main per-layer loop ========
            def main_layer(li, epilogue, mid_cb=None):
                table = tables[li]
                PB = cfg.PIECE_BLOCKS
                for pi, (b0, nb) in enumerate(cfg.pieces):
                    NCh = nb * cfg.CPB
                    # feature gather. SWDGE cost is ~994ns fixed per
                    # instruction + 0.34ns/descriptor, so batching chunks
                    # into one indirect DMA cuts GpSimd issue time. Unused
                    # tail chunks gather row 0: zero-weighted junk.
                    gt = gtA if pi % 2 == 0 else gtB
                    if GATHER_MODE == "hwloop":
                        # hardware loop: one decoded indirect-DMA replayed
                        # NCh times with register-stepped offsets (cuts the
                        # per-instruction sequencer fetch/dispatch cost)
                        with tc.For_i(0, NCh) as j:
                            nc.gpsimd.indirect_dma_start(
                                out=gt[:, ds(j * ROW, ROW)], out_offset=None,
                                in_=table[:, :],
                                in_offset=IndirectOffsetOnAxis(
                                    ap=srcsb[:, ds(b0 * cfg.CPB + j, 1)],
                                    axis=0))
                    elif GATHER_MODE == "piece":
                        off = srcsb[:, b0 * cfg.CPB:(b0 + nb) * cfg.CPB]
                        dstv = gt[:, 0:ROW * NCh].rearrange(
                            "p (j r) -> p j r", r=ROW)
                        nc.gpsimd.indirect_dma_start(
                            out=dstv, out_offset=None,
                            in_=table[:, :],
                            in_offset=IndirectOffsetOnAxis(ap=off, axis=0))
                    elif GATHER_MODE == "block":
                        for bb in range(nb):
                            j0 = bb * cfg.CPB
                            off = srcsb[:, (b0 + bb) * cfg.CPB:
                                        (b0 + bb + 1) * cfg.CPB]
                            dstv = gt[:, ROW * j0:ROW * (j0 + cfg.CPB)] \
                                .rearrange("p (j r) -> p j r", r=ROW)
                            nc.gpsimd.indirect_dma_start(
                                out=dstv, out_offset=None,
                                in_=table[:, :],
                                in_offset=IndirectOffsetOnAxis(ap=off, axis=0))
                    else:  # per-chunk (original)
                        for j in range(NCh):
                            if (j % cfg.CPB) >= used_chunks[b0 + j // cfg.CPB]:
                                continue
                            nc.gpsimd.indirect_dma_start(
                                out=gt[:, ROW * j:ROW * (j + 1)],
                                out_offset=None,
                                in_=table[:, :],
                                in_offset=IndirectOffsetOnAxis(
                                    ap=srcsb[:, b0 * cfg.CPB + j:
                                             b0 * cfg.CPB + j + 1],
                                    axis=0))
                    gv = gt[:, 0:ROW * NCh].rearrange(
                        "p (j c) -> p j c", c=ROW)
                    # d_dst per slot: broadcast slot-0 rows' d across the 8
                    # slot positions with one 16-contraction matmul pair
                    pd = pp.tile([128, PB * cfg.CPB], F32, tag="pd")
                    gvt = gt[0:16, 0:ROW * NCh].rearrange(
                        "p (j c) -> p j c", c=ROW)
                    nc.tensor.matmul(
                        pd[:, 0:NCh], lhsT=mask16t[:],
                        rhs=gvt[:, :, HID + 2:HID + 3].rearrange(
                            "p j c -> p (j c)"),
                        start=True, stop=False)
                    nc.tensor.matmul(
                        pd[:, 0:NCh], lhsT=mask16t[:],
                        rhs=gvt[:, :, HID + 4:HID + 5].rearrange(
                            "p j c -> p (j c)"),
                        start=False, stop=True)
                    # phase A: ex = exp(leaky_relu(s_src + d_dst)) * valid
                    sf = wp.tile([128, PB * cfg.CPB], F32, tag="sf")
                    sfv = sf[:, 0:NCh].rearrange("p j -> p j ()")
                    nc.vector.tensor_tensor(
                        sfv, gv[:, :, HID + 1:HID + 2],
                        gv[:, :, HID + 3:HID + 4], op=OP.add)
                    ep = wp.tile([128, PB * cfg.CPB], F32, tag="ep")
                    nc.vector.tensor_tensor(ep[:, 0:NCh], sf[:, 0:NCh],
                                            pd[:, 0:NCh], op=OP.add)
                    es = wp.tile([128, PB * cfg.CPB], F32, tag="es")
                    nc.vector.tensor_scalar_mul(es[:, 0:NCh], ep[:, 0:NCh],
                                                0.2)
                    el = wp.tile([128, PB * cfg.CPB], F32, tag="el")
                    nc.vector.tensor_tensor(el[:, 0:NCh], ep[:, 0:NCh],
                                            es[:, 0:NCh], op=OP.max)
                    exf = wp.tile([128, PB * cfg.CPB], F32, tag="exf")
                    nc.scalar.activation(exf[:, 0:NCh], el[:, 0:NCh], AF.Exp)
                    exb = wp.tile([128, PB * cfg.CPB], BF, tag="exb")
                    nc.vector.tensor_tensor(
                        exb[:, 0:NCh], exf[:, 0:NCh],
                        validsb[:, b0 * cfg.CPB:b0 * cfg.CPB + NCh],
                        op=OP.mult)
                    # selector build: even chunks -> cols [48a, 48a+16),
                    # odd chunks -> cols [48a+32, 48a+48)
                    sel1 = selA if (b0 // cfg.PIECE_BLOCKS) % 2 == 0 else selB
                    npair = NCh // 2
                    exv = exb[:, 0:NCh].rearrange("p (a u) -> p a u", u=2)
                    maskv = mask16[:].rearrange("p l -> p () l") \
                        .to_broadcast([128, npair, 16])
                    selv = sel1[:, 0:48 * npair].rearrange(
                        "p (a w) -> p a w", w=48)
                    nc.vector.tensor_tensor(
                        selv[:, :, 0:16],
                        exv[:, :, 0:1].to_broadcast([128, npair, 16]),
                        maskv, op=OP.mult)
                    nc.vector.tensor_tensor(
                        selv[:, :, 32:48],
                        exv[:, :, 1:2].to_broadcast([128, npair, 16]),
                        maskv, op=OP.mult)
                    # level 1 + level 2
                    for bb in range(nb):
                        b = b0 + bb
                        pl2 = pp.tile([128, RHSW], F32, tag="l2")
                        for t in range(cfg.TPB):
                            pl1 = pp.tile([128, RHSW], F32, tag="l1")
                            for al in range(4):
                                for u in range(2):
                                    j = bb * cfg.CPB + t * 8 + 2 * al + u
                                    A = j // 2
                                    nc.tensor.matmul(
                                        pl1[32 * al:32 * al + 32, :],
                                        lhsT=sel1[:, 48 * A + 16 * u:
                                                  48 * A + 16 * u + 32],
                                        rhs=gt[:, ROW * j:ROW * j + RHSW],
                                        start=(u == 0), stop=(u == 1),
                                        tile_position=(0, 32 * al))
                            gs = sp.tile([128, RHSW], F32, tag="gsum")
                            nc.scalar.activation(gs[:], pl1[:], AF.Copy)
                            T = b * cfg.TPB + t
                            l2s = wp.tile([128, 128], F32, tag="l2s")
                            nc.vector.tensor_tensor(
                                l2s[:],
                                glabsb[:, T:T + 1].to_broadcast([128, 128]),
                                iota2[:], op=OP.is_equal)
                            nc.tensor.matmul(pl2[:], lhsT=l2s[:], rhs=gs[:],
                                             start=(t == 0),
                                             stop=(t == cfg.TPB - 1))
                        epilogue(b, pl2)
                    if mid_cb is not None:
                        mid_cb(pi)

            def epi_norm(pl2, brow):
                den = wp.tile([128, 1], F32, tag="den")
                nc.vector.tensor_scalar_max(den[:], pl2[:, HID:HID + 1],
                                            1e-30)
                rec = wp.tile([128, 1], F32, tag="rec")
                nc.vector.reciprocal(rec[:], den[:])
                hb = wp.tile([128, HID], F32, tag="hb")
                nc.vector.tensor_scalar_mul(hb[:], pl2[:, 0:HID],
                                            rec[:, 0:1])
                nc.vector.tensor_tensor(hb[:], hb[:], brow[:], op=OP.add)
                return hb

            def epilogue1(b, pl2):
                # fused layer-2 prep: h2 -> [h'|s|d] rows for block b, so
                # only the AllGather sits between the two main loops
                hb = epi_norm(pl2, b1r)
                h2 = wp.tile([128, HID], F32, tag="h2")
                nc.scalar.activation(h2[:], hb[:], AF.Relu)
                tp = pp.tile([128, 128], F32, tag="mm")
                nc.tensor.transpose(tp[:], h2[:], ident[:])
                h2T = wp.tile([128, 128], F32, tag="h2T")
                nc.scalar.activation(h2T[:], tp[:], AF.Copy)
                ppre = pp.tile([128, HID + 2], F32, tag="mm")
                nc.tensor.matmul(ppre[:], lhsT=h2T[:], rhs=rhs2[0][:],
                                 start=True, stop=True)
                build_rows(ppre, b, shards[1])

            def epilogue2(b, pl2):
                hb = epi_norm(pl2, b2r)
                h3 = wp.tile([128, HID], F32, tag="h3")
                nc.scalar.activation(h3[:], hb[:], AF.Relu)
                tp = pp.tile([128, 128], F32, tag="mm")
                nc.tensor.transpose(tp[:], h3[:], ident[:])
                h3T = wp.tile([128, 128], F32, tag="h3T")
                nc.scalar.activation(h3T[:], tp[:], AF.Copy)
                po = pp.tile([128, NCLS], F32, tag="mm")
                nc.tensor.matmul(po[:], lhsT=h3T[:], rhs=wosb[:],
                                 start=True, stop=True)
                lg = wp.tile([128, NCLS], F32, tag="lg")
                nc.vector.tensor_tensor(lg[:], po[:], bor[:], op=OP.add)
                mx = wp.tile([128, 1], F32, tag="mx")
                nc.vector.tensor_reduce(mx[:], lg[:],
                                        axis=mybir.AxisListType.X, op=OP.max)
                lgs = wp.tile([128, NCLS], F32, tag="lgs")
                nc.vector.tensor_scalar(lgs[:], lg[:], mx[:, 0:1], None,
                                        op0=OP.subtract)
                pe = wp.tile([128, NCLS], F32, tag="pe")
                rs = wp.tile([128, 1], F32, tag="rs")
                nc.scalar.activation(pe[:], lgs[:], AF.Exp,
                                     accum_out=rs[:, 0:1])
                rr = wp.tile([128, 1], F32, tag="rr")
                nc.vector.reciprocal(rr[:], rs[:])
                ot = wp.tile([128, NCLS], F32, tag="ot")
                nc.vector.tensor_scalar_mul(ot[:], pe[:], rr[:, 0:1])
                nc.sync.dma_start(out_t[b * 128:(b + 1) * 128, :], ot[:])

            # ======== schedule ========
            # ag half-0s fire early (inside prep_layer1 / mid main-loop) so
            # only the half-1 collectives sit on the critical path.
            AG1_PIECE = max(0, min(len(cfg.pieces) - 2,
                                   (AG_SPLIT_BLOCK // cfg.PIECE_BLOCKS) + 3))

            def mid1(pi):
                if pi == AG1_PIECE:
                    allgather_half(1, 0)

            prep_layer1()
            allgather_half(0, 1)
            main_layer(0, epilogue1, mid_cb=mid1)
            allgather_half(1, 1)
            main_layer(1, epilogue2)

    nc.compile()
    return nc


# ------------------------------------------------------------------ run ---
_PROG_CACHE = {}


def _get_program(cfg, used_chunks=None):
    key = (cfg.N, cfg.E, cfg.NCORES, used_chunks)
    if key not in _PROG_CACHE:
        _PROG_CACHE[key] = build_program(cfg, used_chunks)
    return _PROG_CACHE[key]


_PRE_CACHE = {}


def run(cfg, inputs, trace=False, tmpdir=None):
    from concourse.bass_utils import run_bass_kernel_spmd

    ei = np.asarray(inputs["edge_index"])
    pkey = (cfg.N, cfg.E, int(ei[0, :16].sum()), int(ei[1, -16:].sum()))
    if pkey not in _PRE_CACHE:
        _PRE_CACHE[pkey] = preprocess(cfg, ei)
    pre = _PRE_CACHE[pkey]
    nc = _get_program(cfg, _used_chunks(cfg, pre))
    in_maps = _make_in_maps(cfg, inputs, pre)
    kw = {}
    if trace:
        kw = dict(trace=True, tmpdir=tmpdir)
    res = run_bass_kernel_spmd(nc, in_maps, core_ids=list(range(cfg.NCORES)),
                               **kw)
    outs = []
    for k in range(cfg.NCORES):
        o = res.results[k]["out"][:cfg.V]
        oo = np.empty_like(o)
        oo[pre[k]["perm"]] = o                   # undo block-balancing perm
        outs.append(oo)
    full = np.concatenate(outs, axis=0).astype(np.float32)
    return full, res


def timed_run(cfg, inputs, iters=3):
    """Time device execution with device-resident inputs (axon transfer
    excluded). Returns (best_seconds, outputs_of_last_iter)."""
    import time

    import jax
    from jax.sharding import Mesh, PartitionSpec
    from jax.experimental.shard_map import shard_map
    import concourse.mybir as mybir
    from concourse import bass2jax
    from concourse.bass2jax import _bass_exec_p, partition_id_tensor

    bass2jax.install_neuronx_cc_hook()
    ei = np.asarray(inputs["edge_index"])
    pkey = (cfg.N, cfg.E, int(ei[0, :16].sum()), int(ei[1, -16:].sum()))
    if pkey not in _PRE_CACHE:
        _PRE_CACHE[pkey] = preprocess(cfg, ei)
    pre = _PRE_CACHE[pkey]
    nc = _get_program(cfg, _used_chunks(cfg, pre))
    in_maps = _make_in_maps(cfg, inputs, pre)

    partition_name = (nc.partition_id_tensor.name
                      if nc.partition_id_tensor else None)
    in_names, out_names, out_avals, zero_outs = [], [], [], []
    for alloc in nc.m.functions[0].allocations:
        if not isinstance(alloc, mybir.MemoryLocationSet):
            continue
        name = alloc.memorylocations[0].name
        if alloc.kind == "ExternalInput":
            if name != partition_name:
                in_names.append(name)
        elif alloc.kind == "ExternalOutput":
            out_names.append(name)
            shape = tuple(alloc.tensor_shape)
            dtype = mybir.dt.np(alloc.dtype)
            out_avals.append(jax.core.ShapedArray(shape, dtype))
            zero_outs.append(np.zeros(shape, dtype))
    n_params = len(in_names)
    n_outs = len(out_avals)
    all_in_names = list(in_names) + list(out_names)
    if partition_name is not None:
        all_in_names.append(partition_name)
    donate = tuple(range(n_params, n_params + n_outs))

    def _body(*args):
        operands = list(args)
        if partition_name is not None:
            operands.append(partition_id_tensor())
        outs = _bass_exec_p.bind(
            *operands, out_avals=tuple(out_avals),
            in_names=tuple(all_in_names), out_names=tuple(out_names),
            lowering_input_output_aliases=(),
            sim_require_finite=True, sim_require_nnan=True, nc=nc)
        return tuple(outs)

    devices = jax.devices()[:cfg.NCORES]
    mesh = Mesh(np.asarray(devices), ("core",))
    in_specs = (PartitionSpec("core"),) * (n_params + n_outs)
    out_specs = (PartitionSpec("core"),) * len(out_names)
    sharded = jax.jit(
        shard_map(_body, mesh=mesh, in_specs=in_specs, out_specs=out_specs,
                  check_rep=False),
        donate_argnums=donate, keep_unused=True)
    concat_in = [
        np.concatenate([np.asarray(in_maps[c][nm]) for c in range(cfg.NCORES)],
                       axis=0)
        for nm in in_names]
    sharding = jax.sharding.NamedSharding(mesh, PartitionSpec("core"))
    dev_in = [jax.device_put(a, sharding) for a in concat_in]
    times = []
    out_arrs = None
    for _ in range(iters):
        zo = [jax.device_put(
            np.zeros((cfg.NCORES * z.shape[0], *z.shape[1:]), z.dtype),
            sharding) for z in zero_outs]
        jax.block_until_ready(zo)
        t0 = time.time()
        out_arrs = sharded(*dev_in, *zo)
        jax.block_until_ready(out_arrs)
        times.append(time.time() - t0)
    oi = out_names.index("out")
    full = np.asarray(out_arrs[oi]).reshape(cfg.NCORES, cfg.VPAD, cfg.NCLS)
    outs = []
    for c in range(cfg.NCORES):
        o = full[c, :cfg.V]
        oo = np.empty_like(o)
        oo[pre[c]["perm"]] = o
        outs.append(oo)
    out = np.concatenate(outs, axis=0)
    return min(times), out.astype(np.float32)


def _make_in_maps(cfg, inputs, pre):
    x = np.asarray(inputs["x"], np.float32)
    common = {
        "W1": np.asarray(inputs["W1"], np.float32),
        "W2": np.asarray(inputs["W2"], np.float32),
        "Wo": np.asarray(inputs["Wo"], np.float32),
        "a1s": np.asarray(inputs["a1_src"], np.float32),
        "a1d": np.asarray(inputs["a1_dst"], np.float32),
        "a2s": np.asarray(inputs["a2_src"], np.float32),
        "a2d": np.asarray(inputs["a2_dst"], np.float32),
        "b1": np.asarray(inputs["b1"], np.float32),
        "b2": np.asarray(inputs["b2"], np.float32),
        "bo": np.asarray(inputs["bo"], np.float32),
    }
    in_maps = []
    for k in range(cfg.NCORES):
        xs = x[k * cfg.V:(k + 1) * cfg.V][pre[k]["perm"]]
        xT = np.zeros((cfg.N_IN, cfg.VPAD), BF16)
        xT[:, :cfg.V] = xs.T.astype(BF16)
        m = dict(common)
        m["xT"] = np.ascontiguousarray(xT)
        if GATHER_MODE == "block" and HW_IDX_ORDER:
            m["src_slot"] = _wrap_idx_for_hw(pre[k]["src_slot"], cfg.CPB)
        else:
            m["src_slot"] = pre[k]["src_slot"]
        m["glabel"] = pre[k]["glabel"]
        m["valid"] = pre[k]["valid"]
        in_maps.append(m)
    return in_maps


def kernel(**inputs):
    cfg = DEFAULT_CFG
    full, _ = run(cfg, inputs, trace=False)
    return full

